# revision 1
# baseline (speedup 1.0000x reference)
"""Trainium2 8-core Bass kernel for nn_Attention_7112465842253.

Token-sharded attention: 512 tokens/core (cores 0-3 = batch 0, 4-7 = batch 1).
Per core: QKV projection in bf16 (q/k in transposed [chan, tok] layout, v in
natural [tok, chan]), RMSNorm via weighted-sumsq matmul + Ln/Exp rstd folded
into the RoPE tables, RoPE as x*C + (P@x)*S with a PE partition-swap matmul,
AllGather of K/V inside each 4-core batch group, non-causal attention in
scoresT layout (k-tokens on partitions; softmax denominator from a ones
column appended to V; Exp fused into the PSUM->SBUF eviction on ScalarE),
then the output projection. Host does layout prep and reassembly.
"""

import numpy as np

B, N, DIN, DIM, H, HD = 2, 2048, 1024, 1024, 16, 64
NCORE = 8
TOK = 512
EPS = 1e-6
BASE = 10000.0
KC = N // 128        # 16 k-token chunks
NHG = 4              # head groups of 4

_CACHE = {}


def _build_nc(dbg=None, single=False, zero_bias=False):
    import concourse.bass as bass
    import concourse.tile as tile
    from concourse import bacc, mybir
    from contextlib import ExitStack

    BF = mybir.dt.bfloat16
    F32 = mybir.dt.float32
    AF = mybir.ActivationFunctionType

    nc = bacc.Bacc(
        "TRN2", target_bir_lowering=False, debug=False,
        num_devices=(1 if single else NCORE),
    )

    # ---------------- DRAM parameters ----------------
    xT = nc.dram_tensor("xT", [DIN, TOK], BF, kind="ExternalInput")
    wqkv = nc.dram_tensor("wqkv", [DIN, 3 * DIM], BF, kind="ExternalInput")
    bqkv = nc.dram_tensor("bqkv", [1, 3 * DIM], BF, kind="ExternalInput")
    wsum = nc.dram_tensor("wsum", [DIN, 2], BF, kind="ExternalInput")
    swp = nc.dram_tensor("swp", [128, 128], BF, kind="ExternalInput")
    tabs = nc.dram_tensor("tabs", [4, 128, TOK], BF, kind="ExternalInput")
    wout = nc.dram_tensor("wout", [DIM, DIN], BF, kind="ExternalInput")
    out = nc.dram_tensor("out", [DIN, TOK], F32, kind="ExternalOutput")
    dbgt = (nc.dram_tensor("dbg", [DIN, TOK], F32, kind="ExternalOutput")
            if dbg else None)

    # internal DRAM
    agin = nc.dram_tensor("agin", [2048 * TOK], BF)   # khatT flat + v flat
    agout = nc.dram_tensor("agout", [4 * 2048 * TOK], BF)
    sescr = nc.dram_tensor("sescr", [1, 4 * TOK], F32)  # recip bcast scratch
    sescr2 = nc.dram_tensor("sescr2", [1, 4 * TOK], F32)  # sumexp staging

    RG = [[0, 1, 2, 3], [4, 5, 6, 7]]

    with tile.TileContext(nc) as tc, ExitStack() as CTX:
        # ---------------- persistent SBUF ----------------
        pp = CTX.enter_context(tc.tile_pool(name="persist", bufs=1))
        qhat = [pp.tile([128, TOK], BF, tag=f"qhat{c}", name=f"qhat{c}") for c in range(8)]
        ktf = [pp.tile([128, N], BF, tag=f"ktf{c}", name=f"ktf{c}") for c in range(8)]
        vaug = [pp.tile([128, 16 * 65], BF, tag=f"vaug{c}", name=f"vaug{c}") for c in range(KC)]
        attnT = [pp.tile([128, TOK], BF, tag=f"attnT{c}", name=f"attnT{c}") for c in range(8)]

        bias_sb = pp.tile([1, 3 * DIM], BF, tag="bias_sb", name="bias_sb")
        wsum_sb = pp.tile([128, 8, 2], BF, tag="wsum_sb", name="wsum_sb")
        swp_sb = pp.tile([128, 128], BF, tag="swp_sb", name="swp_sb")
        ones_t = pp.tile([1, TOK], BF, tag="ones_t", name="ones_t")
        ones_c = pp.tile([1, 128], BF, tag="ones_c", name="ones_c")
        eps_sb = pp.tile([1, 1], F32, tag="eps_sb", name="eps_sb")

        nc.vector.memset(ones_t[:], 1.0)
        nc.vector.memset(ones_c[:], 1.0)
        nc.vector.memset(eps_sb[:], EPS)

        # ---------------- phase 1: qkv + norm + rope + AG ----------------
        p1 = ExitStack()
        wq_pool = p1.enter_context(tc.tile_pool(name="wq", bufs=4))
        vw_pool = p1.enter_context(tc.tile_pool(name="vw", bufs=1))
        sq_pool = p1.enter_context(tc.tile_pool(name="sq", bufs=4))
        qtsb_pool = p1.enter_context(tc.tile_pool(name="qtsb", bufs=9))
        scr1_pool = p1.enter_context(tc.tile_pool(name="scr1", bufs=4))
        kvloc_pool = p1.enter_context(tc.tile_pool(name="kvloc", bufs=1))
        rstd_pool = p1.enter_context(tc.tile_pool(name="rstd", bufs=4))
        # PSUM budget (8 banks): qkvps 3 + swpp 3 + ssqp 1 + bcp 1
        qkvp = p1.enter_context(tc.tile_pool(name="qkvp", bufs=3, space="PSUM"))
        swpp = p1.enter_context(tc.tile_pool(name="swpp", bufs=3, space="PSUM"))
        ssqp = p1.enter_context(tc.tile_pool(name="ssqp", bufs=1, space="PSUM"))
        bcp = p1.enter_context(tc.tile_pool(name="bcp", bufs=1, space="PSUM"))
        misc1_pool = p1.enter_context(tc.tile_pool(name="misc1", bufs=1))
        xt_sb = misc1_pool.tile([128, 8, TOK], BF, tag="xt_sb", name="xt_sb")
        tab_r = [misc1_pool.tile([128, TOK], BF, tag=f"tabr{j}", name=f"tabr{j}") for j in range(4)]
        tab_f = [misc1_pool.tile([128, TOK], BF, tag=f"tabf{j}", name=f"tabf{j}") for j in range(4)]
        for ci in range(8):
            nc.scalar.dma_start(
                out=xt_sb[:, ci, :],
                in_=xT.ap()[ci * 128:(ci + 1) * 128, :],
            )
        nc.scalar.dma_start(out=bias_sb[:], in_=bqkv.ap())
        nc.scalar.dma_start(
            out=wsum_sb[:], in_=wsum.ap().rearrange("(c p) w -> p c w", p=128)
        )
        nc.scalar.dma_start(out=swp_sb[:], in_=swp.ap())
        for j in range(4):
            nc.scalar.dma_start(out=tab_r[j][:], in_=tabs.ap()[j])

        def qkv_chunk_psum(co):
            """psum[128ch, TOK] = sum_ci W[ci, co].T @ xT[ci] + b[co] (x) ones."""
            ps = qkvp.tile([128, TOK], F32, tag="qkvps", name="qkvps")
            wt = wq_pool.tile([128, 8, 128], BF, tag="wt", name="wt")
            nc.sync.dma_start(
                out=wt[:],
                in_=wqkv.ap()[:, co * 128:(co + 1) * 128].rearrange(
                    "(c p) m -> p c m", p=128
                ),
            )
            for ci in range(8):
                nc.tensor.matmul(
                    ps[:], wt[:, ci, :], xt_sb[:, ci, :],
                    start=(ci == 0), stop=(zero_bias and ci == 7),
                )
            if zero_bias:
                pass
            else:
                nc.tensor.matmul(
                    ps[:], bias_sb[:, co * 128:(co + 1) * 128], ones_t[:],
                    start=False, stop=True,
                )
            return ps

        def norm_rope_tensor(which, dst_tiles):
            """which: 0 -> q, 1 -> k. Writes 8 roped bf16 chunks to dst_tiles."""
            co0 = which * 8
            qt_list = []
            ssq = ssqp.tile([1, TOK], F32, tag="ssq", name="ssq")
            for c in range(8):
                ps = qkv_chunk_psum(co0 + c)
                qt = qtsb_pool.tile([128, TOK], BF, tag="qt", name="qt")
                nc.vector.tensor_copy(qt[:], ps[:])
                sqt = sq_pool.tile([128, TOK], BF, tag="sqt", name="sqt")
                nc.vector.tensor_mul(sqt[:], qt[:], qt[:])
                nc.tensor.matmul(
                    ssq[:], wsum_sb[:, c, which:which + 1], sqt[:],
                    start=(c == 0), stop=(c == 7),
                )
                qt_list.append(qt)
            # rstd = exp(-0.5 * ln(var + eps))
            lnv = rstd_pool.tile([1, TOK], F32, tag="lnv", name="lnv")
            nc.scalar.activation(lnv[:], ssq[:], AF.Ln, bias=eps_sb[:])
            rstd = rstd_pool.tile([1, TOK], BF, tag="rstd", name="rstd")
            nc.scalar.activation(rstd[:], lnv[:], AF.Exp, scale=-0.5)
            bc = bcp.tile([128, TOK], F32, tag="bc", name="bc")
            nc.tensor.matmul(bc[:], ones_c[:], rstd[:], start=True, stop=True)
            for j in range(2):
                nc.vector.tensor_mul(
                    tab_f[2 * which + j][:], tab_r[2 * which + j][:], bc[:]
                )
            # rope: dst = qt * C' + (P @ qt) * S'
            for c in range(8):
                sw = swpp.tile([128, TOK], F32, tag="sw", name="sw")
                nc.tensor.matmul(
                    sw[:], swp_sb[:], qt_list[c][:], start=True, stop=True
                )
                m1 = scr1_pool.tile([128, TOK], BF, tag="m1", name="m1")
                nc.vector.tensor_mul(m1[:], qt_list[c][:], tab_f[2 * which][:])
                m2 = scr1_pool.tile([128, TOK], BF, tag="m2", name="m2")
                nc.vector.tensor_mul(m2[:], sw[:], tab_f[2 * which + 1][:])
                nc.vector.tensor_add(dst_tiles[c][:], m1[:], m2[:])
            return qt_list

        vw = vw_pool.tile([128, 8, 2 * TOK], BF, tag="vw", name="vw")
        for ci in range(8):
            nc.sync.dma_start(
                out=vw[:, ci, :],
                in_=wqkv.ap()[ci * 128:(ci + 1) * 128, 2 * DIM:],
            )
        for t4 in range(4):
            for nh in range(2):
                ps = qkvp.tile([128, TOK], F32, tag="qkvps", name="qkvps")
                for ci in range(8):
                    nc.tensor.matmul(
                        ps[:],
                        xt_sb[:, ci, t4 * 128:(t4 + 1) * 128],
                        vw[:, ci, nh * TOK:(nh + 1) * TOK],
                        start=(ci == 0), stop=(zero_bias and ci == 7),
                    )
                if not zero_bias:
                    nc.tensor.matmul(
                        ps[:], ones_c[:],
                        bias_sb[:, 2 * DIM + nh * TOK: 2 * DIM + (nh + 1) * TOK],
                        start=False, stop=True,
                    )
                vl = kvloc_pool.tile([128, TOK], BF, tag=f"vloc{t4}_{nh}", name=f"vloc{t4}_{nh}")
                nc.vector.tensor_copy(vl[:], ps[:])
                dstap = bass.AP(
                    tensor=agin.ap().tensor,
                    offset=1024 * TOK + t4 * 128 * 1024 + nh * TOK,
                    ap=[[1024, 128], [1, TOK]],
                )
                nc.gpsimd.dma_start(out=dstap, in_=vl[:])

        # ---- k first (gates scores), AG-k; v overlaps AG-k; then AG-v, q.
        khat = [kvloc_pool.tile([128, TOK], BF, tag=f"khat{c}", name=f"khat{c}") for c in range(8)]
        norm_rope_tensor(1, khat)
        for c in range(8):
            dstap = bass.AP(
                tensor=agin.ap().tensor,
                offset=c * 128 * TOK,
                ap=[[TOK, 128], [1, TOK]],
            )
            nc.gpsimd.dma_start(out=dstap, in_=khat[c][:])

        if single:
            for r in range(4):
                nc.gpsimd.dma_start(
                    out=bass.AP(tensor=agout.ap().tensor,
                                offset=r * 2048 * TOK, ap=[[1, 2048 * TOK]]),
                    in_=bass.AP(tensor=agin.ap().tensor, offset=0,
                                ap=[[1, 2048 * TOK]]),
                )
        else:
            nc.gpsimd.collective_compute(
                "AllGather",
                mybir.AluOpType.bypass,
                replica_groups=RG,
                ins=[agin.ap().opt()],
                outs=[agout.ap().opt()],
            )

        qt_dbg = norm_rope_tensor(0, qhat)

        def dump8(tiles, cols=None):
            dmp = ExitStack()
            dp = dmp.enter_context(tc.tile_pool(name="dump", bufs=2))
            for c, t in enumerate(tiles):
                f = dp.tile([t.shape[0], TOK], F32, tag="dmp", name="dmp")
                srcap = t[:, cols] if cols is not None else t[:]
                nc.vector.tensor_copy(f[:], srcap)
                nc.gpsimd.dma_start(
                    out=dbgt.ap()[c * 128:c * 128 + t.shape[0], :], in_=f[:]
                )
            dmp.close()

        if dbg == "qt":
            dump8(qt_dbg)
        if dbg == "qhat":
            dump8(qhat)
        if dbg == "khat":
            dump8(khat)

        # ---- post-AG loads
        RSZ = 2048 * TOK
        for c in range(8):
            for r in range(4):
                srcap = bass.AP(
                    tensor=agout.ap().tensor,
                    offset=r * RSZ + c * 128 * TOK,
                    ap=[[TOK, 128], [1, TOK]],
                )
                nc.sync.dma_start(
                    out=ktf[c][:, r * TOK:(r + 1) * TOK], in_=srcap
                )
        for r in range(4):
            for t4 in range(4):
                vc = r * 4 + t4
                srcap = bass.AP(
                    tensor=agout.ap().tensor,
                    offset=r * RSZ + 1024 * TOK + t4 * 128 * 1024,
                    ap=[[1024, 128], [64, 16], [1, 64]],
                )
                dst = vaug[vc][:].rearrange("p (h c) -> p h c", c=65)
                nc.sync.dma_start(out=dst[:, :, 0:64], in_=srcap)
                nc.vector.memset(dst[:, :, 64:65], 1.0)

        if dbg == "ktf":
            dump8(ktf, cols=slice(0, TOK))
        if dbg == "vaug":
            dump8(vaug[:8], cols=slice(0, TOK))

        p1.close()

        # ---------------- phase 2: attention ----------------
        p2 = ExitStack()
        wo_res_pool = CTX.enter_context(tc.tile_pool(name="wores", bufs=1))
        wo_pool = CTX.enter_context(tc.tile_pool(name="wo", bufs=3))
        # PSUM: scp 2x2 banks + avp 4x1 = 8
        scp = CTX.enter_context(tc.tile_pool(name="scp", bufs=2, space="PSUM"))
        avp = p2.enter_context(tc.tile_pool(name="avp", bufs=4, space="PSUM"))
        expt_pool = p2.enter_context(tc.tile_pool(name="expt", bufs=20))
        nrm_pool = p2.enter_context(tc.tile_pool(name="nrm", bufs=2))
        ascr_pool = p2.enter_context(tc.tile_pool(name="ascr", bufs=8))

        expt = {}
        av_tiles = {}
        NHG2 = 8   # groups of 2 heads

        def emit_av(hg, kc):
            for hh in range(2):
                h = hg * 2 + hh
                if kc == 0:
                    av_tiles[(hg, hh)] = avp.tile(
                        [65, TOK], F32, tag="av", name="av")
                nc.tensor.matmul(
                    av_tiles[(hg, hh)][:],
                    vaug[kc][:, h * 65:(h + 1) * 65],
                    expt[(hg, kc)][:, hh * TOK:(hh + 1) * TOK],
                    start=(kc == 0), stop=(kc == KC - 1),
                )

        def emit_normalize(hg):
            # sumexp rows live at PSUM partition 64; stage at partition 64,
            # reshape via DRAM to [128,16] for a fast DVE reciprocal, then
            # broadcast-load back.
            se = nrm_pool.tile([65, 2 * TOK], F32, tag="se", name="se")
            for hh in range(2):
                nc.vector.tensor_copy(
                    se[64:65, hh * TOK:(hh + 1) * TOK],
                    av_tiles[(hg, hh)][64:65, :],
                )
            nc.gpsimd.dma_start(
                out=bass.AP(tensor=sescr2.ap().tensor, offset=0,
                            ap=[[1, 2 * TOK]]),
                in_=se[64:65, :])
            sew = nrm_pool.tile([128, 8], F32, tag="sew", name="sew")
            nc.scalar.dma_start(
                out=sew[:],
                in_=bass.AP(tensor=sescr2.ap().tensor, offset=0,
                            ap=[[8, 128], [1, 8]]),
            )
            rw = nrm_pool.tile([128, 8], F32, tag="rw", name="rw")
            nc.vector.reciprocal(out=rw[:], in_=sew[:])
            nc.gpsimd.dma_start(
                out=bass.AP(tensor=sescr.ap().tensor, offset=0,
                            ap=[[8, 128], [1, 8]]),
                in_=rw[:],
            )
            rbc = nrm_pool.tile([64, 2 * TOK], BF, tag="rbc", name="rbc")
            bcast_src = bass.AP(
                tensor=sescr.ap().tensor,
                offset=0,
                ap=[[0, 64], [1, 2 * TOK]],
            )
            nc.gpsimd.dma_start(out=rbc[:], in_=bcast_src)  # casting DMA
            for hh in range(2):
                h = hg * 2 + hh
                a = ascr_pool.tile([64, TOK], BF, tag="ascr", name="ascr")
                nc.vector.tensor_mul(
                    a[:], av_tiles[(hg, hh)][0:64, :],
                    rbc[:, hh * TOK:(hh + 1) * TOK],
                )
                nc.gpsimd.dma_start(
                    out=attnT[h // 2][(h % 2) * 64:(h % 2) * 64 + 64, :],
                    in_=a[:],
                )

        outps_tiles = {}
        for hg in range(NHG2 + 1):
            for kc in range(KC):
                if hg >= 1:
                    emit_av(hg - 1, kc)
                if hg < NHG2:
                    sc = scp.tile([128, 2 * TOK], F32, tag="sc", name="sc")
                    for hh in range(2):
                        h = hg * 2 + hh
                        nc.tensor.matmul(
                            sc[:, hh * TOK:(hh + 1) * TOK],
                            ktf[h // 2][(h % 2) * 64:(h % 2) * 64 + 64,
                                        kc * 128:(kc + 1) * 128],
                            qhat[h // 2][(h % 2) * 64:(h % 2) * 64 + 64, :],
                            start=True, stop=True,
                        )
                    e = expt_pool.tile([128, 2 * TOK], BF, tag="expt",
                                       name="expt")
                    nc.scalar.activation(e[:], sc[:], AF.Exp)
                    expt[(hg, kc)] = e
            if hg == 5:
                # prefetch the output-projection weights during attention
                wo_res = wo_res_pool.tile([128, 8, 1024], BF, tag="wores",
                                          name="wores")
                for ci in range(8):
                    nc.sync.dma_start(
                        out=wo_res[:, ci, :],
                        in_=wout.ap()[ci * 128:(ci + 1) * 128, :],
                    )
            if hg >= 1:
                emit_normalize(hg - 1)
            if dbg == "expt0" and hg == 0:
                dump8([expt[(0, kc)] for kc in range(8)], cols=slice(0, TOK))
            if dbg == "av0" and hg == 1:
                dump8([av_tiles[(0, hh)] for hh in range(2)])
        p2.close()

        # ---------------- phase 3: output projection ----------------
        p3 = ExitStack()
        for co in range(8):
            ps = scp.tile([128, TOK], F32, tag="sc", name="outps")
            for ci in range(8):
                nc.tensor.matmul(
                    ps[:], wo_res[:, ci, co * 128:(co + 1) * 128],
                    attnT[ci][:],
                    start=(ci == 0), stop=(ci == 7),
                )
            osb = wo_pool.tile([128, TOK], F32, tag="osb", name="osb")
            nc.vector.tensor_copy(osb[:], ps[:])
            nc.gpsimd.dma_start(out=out.ap()[co * 128:(co + 1) * 128, :], in_=osb[:])
        p3.close()

    nc.compile()
    return nc


def _host_prep(inputs):
    import ml_dtypes

    bf16 = ml_dtypes.bfloat16
    x = np.asarray(inputs["x"], np.float32)
    Wqkv = np.asarray(inputs["Wqkv"], np.float32)
    bqkv = np.asarray(inputs["bqkv"], np.float32)
    qs = np.asarray(inputs["q_scale"], np.float32)
    ks = np.asarray(inputs["k_scale"], np.float32)
    Wout = np.asarray(inputs["Wout"], np.float32)

    p64 = np.concatenate([np.arange(0, 64, 2), np.arange(1, 64, 2)])
    perm = np.concatenate([64 * h + p64 for h in range(H)])

    qsp, ksp = qs[perm], ks[perm]
    Wq = Wqkv[:, :DIM][:, perm] * qsp[None, :]
    Wk = Wqkv[:, DIM:2 * DIM][:, perm] * ksp[None, :]
    Wv = Wqkv[:, 2 * DIM:]
    W = np.concatenate([Wq, Wk, Wv], 1).astype(bf16)
    bq = bqkv[:DIM][perm] * qsp
    bk = bqkv[DIM:2 * DIM][perm] * ksp
    bias = np.concatenate([bq, bk, bqkv[2 * DIM:]])[None, :].astype(bf16)
    wsum = np.stack(
        [1.0 / (DIM * qsp ** 2), 1.0 / (DIM * ksp ** 2)], 1
    ).astype(bf16)

    sw = np.arange(128)
    swap = np.where(sw % 64 < 32, sw + 32, sw - 32)
    P = np.zeros((128, 128), np.float32)
    P[swap, np.arange(128)] = 1.0  # (P.T @ x)[m] = x[swap[m]]
    P = P.astype(bf16)

    inv_freq = 1.0 / (BASE ** (np.arange(0, HD, 2).astype(np.float32) / HD))
    pos = np.maximum(np.arange(N) - 1, 0).astype(np.float32)
    ang = pos[:, None] * inv_freq[None, :]
    cosT, sinT = np.cos(ang).T, np.sin(ang).T           # (32, N)
    C128 = np.tile(cosT, (4, 1))                         # (128, N)
    S128 = np.concatenate([-sinT, sinT, -sinT, sinT], 0)

    in_maps = []
    for core in range(NCORE):
        b, sh = core // 4, core % 4
        t0 = sh * TOK
        xTs = np.ascontiguousarray(x[b, t0:t0 + TOK, :].T).astype(bf16)
        tabs = np.stack([
            C128[:, t0:t0 + TOK] * 0.125,
            S128[:, t0:t0 + TOK] * 0.125,
            C128[:, t0:t0 + TOK],
            S128[:, t0:t0 + TOK],
        ]).astype(bf16)
        in_maps.append({
            "xT": xTs,
            "wqkv": W,
            "bqkv": bias,
            "wsum": wsum,
            "swp": P,
            "tabs": np.ascontiguousarray(tabs),
            "wout": Wout.astype(bf16),
        })
    return in_maps


LAST_EXEC_NS = None


def kernel(**inputs):
    global LAST_EXEC_NS
    import os
    from concourse.bass_utils import run_bass_kernel_spmd

    dbg = os.environ.get("KERNEL_DBG") or None
    zb = bool(np.all(np.asarray(inputs["bqkv"]) == 0))
    key = f"nc{dbg}{zb}"
    if key not in _CACHE:
        _CACHE[key] = _build_nc(dbg, zero_bias=zb)
    nc = _CACHE[key]

    in_maps = _host_prep(inputs)
    trace = bool(int(os.environ.get("KERNEL_TRACE", "0")))
    tmpdir = None
    if trace:
        import tempfile
        import concourse.bass_utils as _bu
        _bu.upload_artifacts = lambda d: d  # keep artifacts local
        tmpdir = tempfile.mkdtemp(prefix="ktrace_")
        print("TRACE DIR:", tmpdir)
    res = run_bass_kernel_spmd(
        nc, in_maps, core_ids=list(range(NCORE)), trace=trace, tmpdir=tmpdir
    )
    LAST_EXEC_NS = res.exec_time_ns
    bout = np.asarray(inputs["bout"], np.float32)
    out = np.empty((B, N, DIN), np.float32)
    for core in range(NCORE):
        b, sh = core // 4, core % 4
        t0 = sh * TOK
        out[b, t0:t0 + TOK, :] = res.results[core]["out"].T
    out += bout[None, None, :]
    return out


def kernel_raw(inputs):
    """Debug helper: run and return the per-core raw [1024, 512] outputs."""
    global LAST_EXEC_NS
    import os
    from concourse.bass_utils import run_bass_kernel_spmd

    dbg = os.environ.get("KERNEL_DBG") or None
    zb = bool(np.all(np.asarray(inputs["bqkv"]) == 0))
    key = f"nc{dbg}{zb}"
    if key not in _CACHE:
        _CACHE[key] = _build_nc(dbg, zero_bias=zb)
    nc = _CACHE[key]
    in_maps = _host_prep(inputs)
    res = run_bass_kernel_spmd(nc, in_maps, core_ids=list(range(NCORE)))
    LAST_EXEC_NS = res.exec_time_ns
    key = "dbg" if dbg else "out"
    return [r[key] for r in res.results]



# revision 44
# speedup vs baseline: 1.1585x; 1.1585x over previous
"""Trainium2 8-core Bass kernel for nn_Attention_7112465842253.

Token-sharded attention: 512 tokens/core (cores 0-3 = batch 0, 4-7 = batch 1).
Per core, all-bf16 matmuls: QKV projection with q/k in transposed [chan, tok]
layout and v in [tok, chan]; RMSNorm via weighted-sumsq matmul with the Ln/Exp
rstd folded into the RoPE tables; RoPE as x*C + (P@x)*S with a PE
partition-swap matmul, the swaps interleaved into the next stream's QKV
chunks so the PE never stalls on the rope drain.  K is projected+roped first
and its AllGather launched early; Q next (it gates the score stream); V last
with a second AllGather that only gates AV.  Attention runs in scoresT layout
(k-tokens on partitions; softmax denominator from a ones column in V; Exp
fused into the PSUM->SBUF eviction on ScalarE).  The Exp stream on the scalar
engine is the critical path: the AV stream trails it by a lag that covers the
V-gather latency and catches up before the stream ends, so the kernel drains
straight into the output projection.  Host does layout prep and reassembly.
"""

import numpy as np

B, N, DIN, DIM, H, HD = 2, 2048, 1024, 1024, 16, 64
NCORE = 8
TOK = 512
EPS = 1e-6
BASE = 10000.0
KC = N // 128        # 16 k-token chunks
NHG2 = 8             # head-groups of 2
AVLAG = 20           # exp-tiles the AV stream initially trails by
CATCHUP = 60         # stream position where AV starts catching up

_CACHE = {}


def _build_nc(dbg=None, single=False, zero_bias=False):
    import concourse.bass as bass
    import concourse.tile as tile
    from concourse import bacc, mybir
    from contextlib import ExitStack

    BF = mybir.dt.bfloat16
    F32 = mybir.dt.float32
    AF = mybir.ActivationFunctionType

    nc = bacc.Bacc(
        "TRN2", target_bir_lowering=False, debug=False,
        num_devices=(1 if single else NCORE),
    )

    # ---------------- DRAM parameters ----------------
    xT = nc.dram_tensor("xT", [DIN, TOK], BF, kind="ExternalInput")
    wqkv = nc.dram_tensor("wqkv", [DIN, 3 * DIM], BF, kind="ExternalInput")
    bqkv = nc.dram_tensor("bqkv", [1, 3 * DIM], BF, kind="ExternalInput")
    wsum = nc.dram_tensor("wsum", [DIN, 2], BF, kind="ExternalInput")
    swp = nc.dram_tensor("swp", [128, 128], BF, kind="ExternalInput")
    tabs = nc.dram_tensor("tabs", [4, 128, TOK], BF, kind="ExternalInput")
    wout = nc.dram_tensor("wout", [DIM, DIN], BF, kind="ExternalInput")
    out = nc.dram_tensor("out", [DIN, TOK], F32, kind="ExternalOutput")
    dbgt = (nc.dram_tensor("dbg", [DIN, TOK], F32, kind="ExternalOutput")
            if dbg else None)

    SHSZ = DIM * TOK                       # one core's k shard, elems
    VROW = 16 * 65                         # v row: 16 heads x (64 chans + 1)
    SHSZV = TOK * VROW                     # one core's augmented v shard
    agk_in = nc.dram_tensor("agk_in", [SHSZ], BF)
    agk_out = nc.dram_tensor("agk_out", [4 * SHSZ], BF)
    agv_in = nc.dram_tensor("agv_in", [SHSZV], BF)
    agv_out = nc.dram_tensor("agv_out", [4 * SHSZV], BF)
    sescr = nc.dram_tensor("sescr", [1, 4 * TOK], BF)   # recip bcast scratch

    RG = [[0, 1, 2, 3], [4, 5, 6, 7]]

    with tile.TileContext(nc) as tc, ExitStack() as CTX:
        # ---------------- persistent SBUF ----------------
        pp = CTX.enter_context(tc.tile_pool(name="persist", bufs=1))
        qhat = [pp.tile([128, TOK], BF, tag=f"qhat{c}", name=f"qhat{c}")
                for c in range(8)]
        ktf = [pp.tile([128, 8, TOK], BF, tag=f"ktf{r}", name=f"ktf{r}")
               for r in range(4)]
        vaug = [[pp.tile([128, 4, VROW // 2], BF, tag=f"vaug{r}_{h}",
                         name=f"vaug{r}_{h}") for h in range(2)]
                for r in range(4)]
        attnT = [pp.tile([128, TOK], BF, tag=f"attnT{c}", name=f"attnT{c}")
                 for c in range(8)]

        bias_sb = pp.tile([1, 3 * DIM], BF, tag="bias_sb", name="bias_sb")
        wsum_sb = pp.tile([128, 8, 2], BF, tag="wsum_sb", name="wsum_sb")
        swp_sb = pp.tile([128, 128], BF, tag="swp_sb", name="swp_sb")
        ones_t = pp.tile([1, TOK], BF, tag="ones_t", name="ones_t")
        ones_c = pp.tile([1, 128], BF, tag="ones_c", name="ones_c")
        eps_sb = pp.tile([1, 1], F32, tag="eps_sb", name="eps_sb")
        tab_r = pp.tile([128, 4, TOK], BF, tag="tab_r", name="tab_r")
        tab_f = pp.tile([128, 4, TOK], BF, tag="tab_f", name="tab_f")

        nc.vector.memset(ones_t[:], 1.0)
        nc.vector.memset(ones_c[:], 1.0)
        nc.vector.memset(eps_sb[:], EPS)

        wo_res_pool = CTX.enter_context(tc.tile_pool(name="wores", bufs=1))

        # ---------------- phase 1: qkv + norm + rope + AG ----------------
        p1 = ExitStack()
        w_pool = p1.enter_context(tc.tile_pool(name="wls", bufs=2))
        sq_pool = p1.enter_context(tc.tile_pool(name="sq", bufs=4))
        qtsb_pool = p1.enter_context(tc.tile_pool(name="qtsb", bufs=12))
        scr1_pool = p1.enter_context(tc.tile_pool(name="scr1", bufs=4))
        rstd_pool = p1.enter_context(tc.tile_pool(name="rstd", bufs=4))
        # PSUM budget (8 banks): qkvp 3 + swpp 3 + ssqp 1 + bcp 1 = 8
        qkvp = p1.enter_context(tc.tile_pool(name="qkvp", bufs=4, space="PSUM"))
        swpp = p1.enter_context(tc.tile_pool(name="swpp", bufs=2, space="PSUM"))
        ssqp = p1.enter_context(tc.tile_pool(name="ssqp", bufs=1, space="PSUM"))
        bcp = p1.enter_context(tc.tile_pool(name="bcp", bufs=1, space="PSUM"))

        misc1_pool = p1.enter_context(tc.tile_pool(name="misc1", bufs=1))
        xt_sb = misc1_pool.tile([128, 8, TOK], BF, tag="xt_sb", name="xt_sb")
        khat = [misc1_pool.tile([128, 4, TOK], BF, tag=f"khat{h}",
                                name=f"khat{h}") for h in range(2)]
        vloc = [misc1_pool.tile([128, 4, VROW // 2], BF, tag=f"vloc{h}",
                                name=f"vloc{h}") for h in range(2)]

        # prefetches: Wk's first column chunk, then x, then the rest of
        # Wk in growing pieces, so the first QKV matmul starts ~4.5us in
        wkt = w_pool.tile([128, 8, DIM], BF, tag="wt", name="wt1")
        for lo, hi in ((0, 128), (None, None), (128, 512), (512, 1024)):
            if lo is None:
                nc.sync.dma_start(
                    out=xt_sb[:],
                    in_=xT.ap().rearrange("(c p) t -> p c t", p=128))
                continue
            nc.sync.dma_start(
                out=wkt[:, :, lo:hi],
                in_=wqkv.ap()[:, DIM + lo:DIM + hi].rearrange(
                    "(c p) m -> p c m", p=128),
            )
        nc.scalar.dma_start(
            out=tab_r[:], in_=tabs.ap().rearrange("j p t -> p j t"))
        nc.scalar.dma_start(out=swp_sb[:], in_=swp.ap())
        nc.scalar.dma_start(
            out=wsum_sb[:], in_=wsum.ap().rearrange("(c p) w -> p c w", p=128))
        if not zero_bias:
            nc.scalar.dma_start(out=bias_sb[:], in_=bqkv.ap())

        def load_w(which):
            """which 0/1/2 -> q/k/v weight block as [128, 8, 1024]."""
            wt = w_pool.tile([128, 8, DIM], BF, tag="wt", name=f"wt{which}")
            nc.sync.dma_start(
                out=wt[:],
                in_=wqkv.ap()[:, which * DIM:(which + 1) * DIM].rearrange(
                    "(c p) m -> p c m", p=128),
            )
            return wt

        wk = wkt
        wq = load_w(0)

        HS = SHSZ // 2

        def store_k_half(h, eng):
            # AllGather-k input, split by head halves: the first half (k
            # chunks 0-3 = heads 0-7) flows store->copy->reload on the
            # gpsimd queue so the score stream starts while the rest moves.
            eng.dma_start(
                out=bass.AP(tensor=agk_in.ap().tensor, offset=h * HS,
                            ap=[[TOK, 128], [128 * TOK, 4], [1, TOK]]),
                in_=khat[h][:],
            )

        def qkv_chunk(wt, which, c, ssq):
            """One 128-channel chunk: psum matmuls, ACT evict, square+ssq."""
            ps = qkvp.tile([128, TOK], F32, tag="qkvps", name="qkvps")
            for ci in range(8):
                nc.tensor.matmul(
                    ps[:], wt[:, ci, c * 128:(c + 1) * 128], xt_sb[:, ci, :],
                    start=(ci == 0), stop=(zero_bias and ci == 7),
                )
            if not zero_bias:
                nc.tensor.matmul(
                    ps[:], bias_sb[:, which * DIM + c * 128:
                                   which * DIM + (c + 1) * 128],
                    ones_t[:], start=False, stop=True,
                )
            qt = qtsb_pool.tile([128, TOK], BF, tag="qt", name="qt")
            nc.scalar.copy(qt[:], ps[:])   # evict on ACT (idle in phase 1)
            sqt = sq_pool.tile([128, TOK], BF, tag="sqt", name="sqt")
            nc.scalar.square(sqt[:], ps[:])
            return qt, sqt

        def rstd_tabs(which, ssq):
            # rstd = exp(-0.5 * ln(var + eps)); fold into the rope tables
            lnv = rstd_pool.tile([1, TOK], F32, tag="lnv", name="lnv")
            nc.scalar.activation(lnv[:], ssq[:], AF.Ln, bias=eps_sb[:])
            rstd = rstd_pool.tile([1, TOK], BF, tag="rstd", name="rstd")
            nc.scalar.activation(rstd[:], lnv[:], AF.Exp, scale=-0.5)
            bc = bcp.tile([128, TOK], F32, tag="bc", name="bc")
            nc.tensor.matmul(bc[:], ones_c[:], rstd[:], start=True, stop=True)
            for j in range(2):
                nc.vector.tensor_mul(
                    tab_f[:, 2 * which + j, :], tab_r[:, 2 * which + j, :],
                    bc[:])

        def rope_chunk(which, qt, dst):
            """dst = qt * C' + (P @ qt) * S'."""
            sw = swpp.tile([128, TOK], F32, tag="sw", name="sw")
            nc.tensor.matmul(sw[:], swp_sb[:], qt[:], start=True, stop=True)
            m1 = scr1_pool.tile([128, TOK], BF, tag="m1", name="m1")
            nc.vector.tensor_mul(m1[:], qt[:], tab_f[:, 2 * which, :])
            m2 = scr1_pool.tile([128, TOK], BF, tag="m2", name="m2")
            nc.vector.tensor_mul(m2[:], sw[:], tab_f[:, 2 * which + 1, :])
            nc.vector.tensor_add(dst, m1[:], m2[:])

        def emit_ssq(ssq, which, c, sqt):
            nc.tensor.matmul(
                ssq[:], wsum_sb[:, c, which:which + 1], sqt[:],
                start=(c == 0), stop=(c == 7),
            )

        # ---- k chunks (ssq matmuls lag 2 chunks so PE never waits the
        # evict->square chain)
        ssq_k = ssqp.tile([1, TOK], F32, tag="ssq", name="ssq_k")
        kt, ksq = [], []
        for c in range(8):
            a, b = qkv_chunk(wk, 1, c, ssq_k)
            kt.append(a)
            ksq.append(b)
            if c >= 2:
                emit_ssq(ssq_k, 1, c - 2, ksq[c - 2])
        emit_ssq(ssq_k, 1, 6, ksq[6])
        emit_ssq(ssq_k, 1, 7, ksq[7])
        rstd_tabs(1, ssq_k)
        # ---- q chunks with k-rope interleaved (PE never idles on the drain)
        wv = load_w(2)      # recycles a w buffer once the k-chunks finish
        ssq_q = ssqp.tile([1, TOK], F32, tag="ssq", name="ssq_q")
        qt, qsq = [], []
        for c in range(8):
            a, b = qkv_chunk(wq, 0, c, ssq_q)
            qt.append(a)
            qsq.append(b)
            if c >= 2:
                emit_ssq(ssq_q, 0, c - 2, qsq[c - 2])
            if c < 4:   # front-load the k-rope: 2 chunks per q-chunk
                rope_chunk(1, kt[2 * c], khat[c // 2][:, (2 * c) % 4, :])
                rope_chunk(1, kt[2 * c + 1], khat[c // 2][:, (2 * c + 1) % 4, :])
                if c == 1:
                    store_k_half(0, nc.gpsimd)
                if c == 3:
                    store_k_half(1, nc.sync)
        emit_ssq(ssq_q, 0, 6, qsq[6])
        emit_ssq(ssq_q, 0, 7, qsq[7])
        rstd_tabs(0, ssq_q)


        def agk_copy(r, h, eng):
            eng.dma_start(
                out=bass.AP(tensor=agk_out.ap().tensor,
                            offset=r * SHSZ + h * HS, ap=[[1, HS]]),
                in_=bass.AP(tensor=agk_in.ap().tensor, offset=h * HS,
                            ap=[[1, HS]]),
            )

        def ktf_reload(r, h, eng):
            srcap = bass.AP(
                tensor=agk_out.ap().tensor, offset=r * SHSZ + h * HS,
                ap=[[TOK, 128], [128 * TOK, 4], [1, TOK]],
            )
            eng.dma_start(out=ktf[r][:, 4 * h:4 * h + 4, :], in_=srcap)

        if single:
            for r in range(4):
                agk_copy(r, 0, nc.gpsimd)
                ktf_reload(r, 0, nc.gpsimd if r < 2 else nc.sync)
            for r in range(4):
                agk_copy(r, 1, nc.sync)
                ktf_reload(r, 1, nc.gpsimd if r % 2 == 0 else nc.sync)
        else:
            nc.gpsimd.collective_compute(
                "AllGather", mybir.AluOpType.bypass, replica_groups=RG,
                ins=[agk_in.ap().opt()], outs=[agk_out.ap().opt()],
            )
            for r in range(4):
                ktf_reload(r, 0, nc.gpsimd if r < 2 else nc.sync)
                ktf_reload(r, 1, nc.gpsimd if r % 2 == 0 else nc.sync)

        # ---- v chunks with q-rope interleaved, nh-outer so the heads-0-7
        # half of the augmented V completes (and gathers) first.  vloc
        # carries the (64 chans + ones) per-head layout so the gather
        # delivers AV-ready tiles with no strided reload.
        vlr = [vloc[h][:].rearrange("p f (g c) -> p f g c", c=65)
               for h in range(2)]
        for h in range(2):
            nc.vector.memset(vlr[h][:, :, :, 64:65], 1.0)

        def store_v_half(h, eng):
            eng.dma_start(
                out=bass.AP(tensor=agv_in.ap().tensor, offset=h * (VROW // 2),
                            ap=[[VROW, 128], [128 * VROW, 4], [1, VROW // 2]]),
                in_=vloc[h][:],
            )

        rope_chunk(0, qt[0], qhat[0][:])
        vi = 0
        for nh in range(2):
            for t4 in range(4):
                ps = qkvp.tile([128, TOK], F32, tag="qkvps", name="qkvps")
                for ci in range(8):
                    nc.tensor.matmul(
                        ps[:],
                        xt_sb[:, ci, t4 * 128:(t4 + 1) * 128],
                        wv[:, ci, nh * TOK:(nh + 1) * TOK],
                        start=(ci == 0), stop=(zero_bias and ci == 7),
                    )
                if not zero_bias:
                    nc.tensor.matmul(
                        ps[:], ones_c[:],
                        bias_sb[:, 2 * DIM + nh * TOK:2 * DIM + (nh + 1) * TOK],
                        start=False, stop=True,
                    )
                dst = vlr[nh][:, t4, :, 0:64]
                nc.vector.tensor_copy(dst, ps[:])
                if vi < 7:
                    rope_chunk(0, qt[vi + 1], qhat[vi + 1][:])
                vi += 1
            store_v_half(nh, nc.sync if nh == 0 else nc.sync)
        wo_res = wo_res_pool.tile([128, 8, DIN], BF, tag="wores", name="wores")
        nc.sync.dma_start(
            out=wo_res[:],
            in_=wout.ap().rearrange("(c p) m -> p c m", p=128),
        )

        def agv_copy(r, h, eng):
            eng.dma_start(
                out=bass.AP(tensor=agv_out.ap().tensor,
                            offset=r * SHSZV + h * (VROW // 2),
                            ap=[[VROW, TOK], [1, VROW // 2]]),
                in_=bass.AP(tensor=agv_in.ap().tensor, offset=h * (VROW // 2),
                            ap=[[VROW, TOK], [1, VROW // 2]]),
            )

        def vaug_reload(r, h, eng):
            srcap = bass.AP(
                tensor=agv_out.ap().tensor, offset=r * SHSZV + h * (VROW // 2),
                ap=[[VROW, 128], [128 * VROW, 4], [1, VROW // 2]],
            )
            eng.dma_start(out=vaug[r][h][:], in_=srcap)

        if single:
            for r in range(4):
                agv_copy(r, 0, nc.gpsimd)
                vaug_reload(r, 0, nc.gpsimd if r < 2 else nc.sync)
            for r in range(4):
                agv_copy(r, 1, nc.sync)
                vaug_reload(r, 1, nc.gpsimd if r % 2 == 0 else nc.sync)
        else:
            nc.gpsimd.collective_compute(
                "AllGather", mybir.AluOpType.bypass, replica_groups=RG,
                ins=[agv_in.ap().opt()], outs=[agv_out.ap().opt()],
            )
            for r in range(4):
                vaug_reload(r, 0, nc.gpsimd if r < 2 else nc.sync)
                vaug_reload(r, 1, nc.gpsimd if r % 2 == 0 else nc.sync)

        def dump8(tiles, cols=None):
            dmp = ExitStack()
            dp = dmp.enter_context(tc.tile_pool(name="dump", bufs=2))
            for c, t in enumerate(tiles):
                f = dp.tile([t.shape[0], TOK], F32, tag="dmp", name="dmp")
                srcap = t[:, cols] if cols is not None else t[:]
                nc.vector.tensor_copy(f[:], srcap)
                nc.gpsimd.dma_start(
                    out=dbgt.ap()[c * 128:c * 128 + t.shape[0], :], in_=f[:])
            dmp.close()

        if dbg == "qt":
            dump8(qt)
        if dbg == "qhat":
            dump8(qhat)
        if dbg == "khat":
            dump8([khat[c // 4][:, c % 4, :] for c in range(8)])
        if dbg == "ktf":
            dump8([ktf[0][:, c, 0:TOK] for c in range(8)])
        if dbg == "vaug":
            dump8([vaug[r][0][:, 0, 0:TOK] for r in range(4)] * 2)

        p1.close()

        # ---------------- phase 2: attention ----------------
        p2 = ExitStack()
        wo_pool = CTX.enter_context(tc.tile_pool(name="wo", bufs=3))
        # PSUM: scp 2x2 banks + avp 4x1 = 8
        scp = CTX.enter_context(tc.tile_pool(name="scp", bufs=2, space="PSUM"))
        avp = p2.enter_context(tc.tile_pool(name="avp", bufs=4, space="PSUM"))
        expt_pool = p2.enter_context(
            tc.tile_pool(name="expt", bufs=AVLAG + 2))
        nrm_pool = p2.enter_context(tc.tile_pool(name="nrm", bufs=2))
        ascr_pool = p2.enter_context(tc.tile_pool(name="ascr", bufs=4))

        av_tiles = {}
        expt = {}

        def emit_score_exp(hg, kc):
            sc = scp.tile([128, 2 * TOK], F32, tag="sc", name="sc")
            for hh in range(2):
                h = hg * 2 + hh
                nc.tensor.matmul(
                    sc[:, hh * TOK:(hh + 1) * TOK],
                    ktf[kc // 4][(h % 2) * 64:(h % 2) * 64 + 64, h // 2,
                                 (kc % 4) * 128:(kc % 4 + 1) * 128],
                    qhat[h // 2][(h % 2) * 64:(h % 2) * 64 + 64, :],
                    start=True, stop=True,
                )
            e = expt_pool.tile([128, 2 * TOK], BF, tag="expt", name="expt")
            nc.scalar.activation(e[:], sc[:], AF.Exp)
            expt[(hg, kc)] = e

        def emit_av(hg, j):
            e = expt.pop((hg, j))
            for hh in range(2):
                if j == 0:
                    av_tiles[(hg, hh)] = avp.tile(
                        [65, TOK], F32, tag="av", name="av")
                ha = hg * 2 + hh
                nc.tensor.matmul(
                    av_tiles[(hg, hh)][:],
                    vaug[j // 4][ha // 8][:, j % 4,
                                 (ha % 8) * 65:(ha % 8 + 1) * 65],
                    e[:, hh * TOK:(hh + 1) * TOK],
                    start=(j == 0), stop=(j == KC - 1),
                )

        def emit_normalize(hg):
            # sumexp rows live at PSUM partition 64; stage both heads' rows
            # into one SBUF row, reciprocal in place, round-trip through DRAM
            # once to broadcast across 64 partitions (stride-0 load), then
            # scale into the attnT slots (odd heads cross a partition offset,
            # which only a DMA can do).
            se = nrm_pool.tile([65, 2 * TOK], F32, tag="se", name="se")
            for hh in range(2):
                nc.vector.tensor_copy(
                    se[64:65, hh * TOK:(hh + 1) * TOK],
                    av_tiles[(hg, hh)][64:65, :],
                )
            nc.vector.reciprocal(out=se[64:65, :], in_=se[64:65, :])
            nc.gpsimd.dma_start(
                out=bass.AP(tensor=sescr.ap().tensor,
                            offset=(hg % 2) * 2 * TOK,
                            ap=[[1, 2 * TOK]]),
                in_=se[64:65, :])   # f32 -> bf16 casting DMA
            rbct = nrm_pool.tile([64, 2 * TOK], BF, tag="rbc", name="rbc")
            bcast_src = bass.AP(
                tensor=sescr.ap().tensor, offset=(hg % 2) * 2 * TOK,
                ap=[[0, 64], [1, 2 * TOK]],
            )
            nc.gpsimd.dma_start(out=rbct[:], in_=bcast_src)
            rbc = rbct[:]
            for hh in range(2):
                h = hg * 2 + hh
                if h % 2 == 0:
                    nc.vector.tensor_mul(
                        attnT[h // 2][0:64, :],
                        av_tiles[(hg, hh)][0:64, :],
                        rbc[0:64, hh * TOK:(hh + 1) * TOK],
                    )
                else:
                    a = ascr_pool.tile([64, TOK], BF, tag="ascr", name="ascr")
                    nc.vector.tensor_mul(
                        a[:], av_tiles[(hg, hh)][0:64, :],
                        rbc[0:64, hh * TOK:(hh + 1) * TOK],
                    )
                    nc.sync.dma_start(
                        out=attnT[h // 2][64:128, :], in_=a[:])

        # Score/exp stream runs ahead; the AV stream trails by AVLAG tiles
        # (covering the V-gather) and catches up after CATCHUP so the stream
        # drains straight into the output projection.
        NT = NHG2 * KC
        av_done = 0

        def av_target(t):
            base = max(0, t - AVLAG + 1)
            extra = max(0, (t - CATCHUP)) // 3
            if t >= NT - 1:
                return NT
            return min(NT, base + extra, t + 1)

        for t in range(NT):
            hg, j = t // KC, t % KC
            emit_score_exp(hg, j)
            while av_done < av_target(t):
                ahg, aj = av_done // KC, av_done % KC
                emit_av(ahg, aj)
                if aj == KC - 1:
                    emit_normalize(ahg)
                av_done += 1
        p2.close()

        # ---------------- phase 3: output projection ----------------
        # Two co-chunks share one PSUM tile; the head-pair whose attnT slot
        # lands last (ci=3, via the hg7 ascr DMA) is contracted last.
        CIORD = [0, 1, 2, 4, 5, 6, 7, 3]
        for cop in range(4):
            ps = scp.tile([128, 2 * TOK], F32, tag="sc", name="outps")
            for sub in range(2):
                co = cop * 2 + sub
                for i, ci in enumerate(CIORD):
                    nc.tensor.matmul(
                        ps[:, sub * TOK:(sub + 1) * TOK],
                        wo_res[:, ci, co * 128:(co + 1) * 128],
                        attnT[ci][:],
                        start=(i == 0), stop=(i == 7),
                    )
            for sub in range(2):
                co = cop * 2 + sub
                osb = wo_pool.tile([128, TOK], F32, tag="osb", name="osb")
                nc.vector.tensor_copy(osb[:], ps[:, sub * TOK:(sub + 1) * TOK])
                nc.sync.dma_start(out=out.ap()[co * 128:(co + 1) * 128, :],
                                  in_=osb[:])

    nc.compile()
    return nc


def _host_prep(inputs):
    import ml_dtypes

    bf16 = ml_dtypes.bfloat16
    x = np.asarray(inputs["x"], np.float32)
    Wqkv = np.asarray(inputs["Wqkv"], np.float32)
    bqkv = np.asarray(inputs["bqkv"], np.float32)
    qs = np.asarray(inputs["q_scale"], np.float32)
    ks = np.asarray(inputs["k_scale"], np.float32)
    Wout = np.asarray(inputs["Wout"], np.float32)

    p64 = np.concatenate([np.arange(0, 64, 2), np.arange(1, 64, 2)])
    perm = np.concatenate([64 * h + p64 for h in range(H)])

    qsp, ksp = qs[perm], ks[perm]
    Wq = Wqkv[:, :DIM][:, perm] * qsp[None, :]
    Wk = Wqkv[:, DIM:2 * DIM][:, perm] * ksp[None, :]
    Wv = Wqkv[:, 2 * DIM:]
    W = np.concatenate([Wq, Wk, Wv], 1).astype(bf16)
    bq = bqkv[:DIM][perm] * qsp
    bk = bqkv[DIM:2 * DIM][perm] * ksp
    bias = np.concatenate([bq, bk, bqkv[2 * DIM:]])[None, :].astype(bf16)
    wsum = np.stack(
        [1.0 / (DIM * qsp ** 2), 1.0 / (DIM * ksp ** 2)], 1
    ).astype(bf16)

    sw = np.arange(128)
    swap = np.where(sw % 64 < 32, sw + 32, sw - 32)
    P = np.zeros((128, 128), np.float32)
    P[swap, np.arange(128)] = 1.0  # (P.T @ x)[m] = x[swap[m]]
    P = P.astype(bf16)

    inv_freq = 1.0 / (BASE ** (np.arange(0, HD, 2).astype(np.float32) / HD))
    pos = np.maximum(np.arange(N) - 1, 0).astype(np.float32)
    ang = pos[:, None] * inv_freq[None, :]
    cosT, sinT = np.cos(ang).T, np.sin(ang).T           # (32, N)
    C128 = np.tile(cosT, (4, 1))                         # (128, N)
    S128 = np.concatenate([-sinT, sinT, -sinT, sinT], 0)

    in_maps = []
    for core in range(NCORE):
        b, sh = core // 4, core % 4
        t0 = sh * TOK
        xTs = np.ascontiguousarray(x[b, t0:t0 + TOK, :].T).astype(bf16)
        tabs = np.stack([
            C128[:, t0:t0 + TOK] * 0.125,
            S128[:, t0:t0 + TOK] * 0.125,
            C128[:, t0:t0 + TOK],
            S128[:, t0:t0 + TOK],
        ]).astype(bf16)
        in_maps.append({
            "xT": xTs,
            "wqkv": W,
            "bqkv": bias,
            "wsum": wsum,
            "swp": P,
            "tabs": np.ascontiguousarray(tabs),
            "wout": Wout.astype(bf16),
        })
    return in_maps


LAST_EXEC_NS = None


def kernel(**inputs):
    global LAST_EXEC_NS
    import os
    from concourse.bass_utils import run_bass_kernel_spmd

    dbg = os.environ.get("KERNEL_DBG") or None
    zb = bool(np.all(np.asarray(inputs["bqkv"]) == 0))
    key = f"nc{dbg}{zb}"
    if key not in _CACHE:
        _CACHE[key] = _build_nc(dbg, zero_bias=zb)
    nc = _CACHE[key]

    in_maps = _host_prep(inputs)
    trace = bool(int(os.environ.get("KERNEL_TRACE", "0")))
    tmpdir = None
    if trace:
        import tempfile
        import concourse.bass_utils as _bu
        _bu.upload_artifacts = lambda d: d  # keep artifacts local
        tmpdir = tempfile.mkdtemp(prefix="ktrace_")
        print("TRACE DIR:", tmpdir)
    res = run_bass_kernel_spmd(
        nc, in_maps, core_ids=list(range(NCORE)), trace=trace, tmpdir=tmpdir
    )
    LAST_EXEC_NS = res.exec_time_ns
    bout = np.asarray(inputs["bout"], np.float32)
    outf = np.empty((B, N, DIN), np.float32)
    for core in range(NCORE):
        b, sh = core // 4, core % 4
        t0 = sh * TOK
        outf[b, t0:t0 + TOK, :] = res.results[core]["out"].T
    outf += bout[None, None, :]
    return outf


def kernel_raw(inputs):
    """Debug helper: run and return the per-core raw [1024, 512] outputs."""
    global LAST_EXEC_NS
    import os
    from concourse.bass_utils import run_bass_kernel_spmd

    dbg = os.environ.get("KERNEL_DBG") or None
    zb = bool(np.all(np.asarray(inputs["bqkv"]) == 0))
    key = f"nc{dbg}{zb}"
    if key not in _CACHE:
        _CACHE[key] = _build_nc(dbg, zero_bias=zb)
    nc = _CACHE[key]
    in_maps = _host_prep(inputs)
    res = run_bass_kernel_spmd(nc, in_maps, core_ids=list(range(NCORE)))
    LAST_EXEC_NS = res.exec_time_ns
    key = "dbg" if dbg else "out"
    return [r[key] for r in res.results]


# revision 47
# speedup vs baseline: 1.1841x; 1.0221x over previous
"""Trainium2 8-core Bass kernel for nn_Attention_7112465842253.

Token-sharded attention: 512 tokens/core (cores 0-3 = batch 0, 4-7 = batch 1).
Per core, all-bf16 matmuls: QKV projection with q/k in transposed [chan, tok]
layout and v in [tok, chan]; RMSNorm via weighted-sumsq matmul with the Ln/Exp
rstd folded into the RoPE tables; RoPE as x*C + (P@x)*S with a PE
partition-swap matmul, the swaps interleaved into the next stream's QKV
chunks so the PE never stalls on the rope drain.  K is projected+roped first
and its AllGather launched early; Q next (it gates the score stream); V last
with a second AllGather that only gates AV.  Attention runs in scoresT layout
(k-tokens on partitions; softmax denominator from a ones column in V; Exp
fused into the PSUM->SBUF eviction on ScalarE).  The Exp stream on the scalar
engine is the critical path: the AV stream trails it by a lag that covers the
V-gather latency and catches up before the stream ends, so the kernel drains
straight into the output projection.  Host does layout prep and reassembly.
"""

import numpy as np

B, N, DIN, DIM, H, HD = 2, 2048, 1024, 1024, 16, 64
NCORE = 8
TOK = 512
EPS = 1e-6
BASE = 10000.0
KC = N // 128        # 16 k-token chunks
NHG2 = 8             # head-groups of 2
AVLAG = 20           # exp-tiles the AV stream initially trails by
CATCHUP = 60         # stream position where AV starts catching up

_CACHE = {}


def _build_nc(dbg=None, single=False, zero_bias=False):
    import concourse.bass as bass
    import concourse.tile as tile
    from concourse import bacc, mybir
    from contextlib import ExitStack

    BF = mybir.dt.bfloat16
    F32 = mybir.dt.float32
    AF = mybir.ActivationFunctionType

    nc = bacc.Bacc(
        "TRN2", target_bir_lowering=False, debug=False,
        num_devices=(1 if single else NCORE),
    )

    # ---------------- DRAM parameters ----------------
    xT = nc.dram_tensor("xT", [DIN, TOK], BF, kind="ExternalInput")
    wqkv = nc.dram_tensor("wqkv", [DIN, 3 * DIM], BF, kind="ExternalInput")
    bqkv = nc.dram_tensor("bqkv", [1, 3 * DIM], BF, kind="ExternalInput")
    wsum = nc.dram_tensor("wsum", [DIN, 2], BF, kind="ExternalInput")
    swp = nc.dram_tensor("swp", [128, 128], BF, kind="ExternalInput")
    tabs = nc.dram_tensor("tabs", [4, 128, TOK], BF, kind="ExternalInput")
    wout = nc.dram_tensor("wout", [DIM, DIN], BF, kind="ExternalInput")
    out = nc.dram_tensor("out", [DIN, TOK], F32, kind="ExternalOutput")
    dbgt = (nc.dram_tensor("dbg", [DIN, TOK], F32, kind="ExternalOutput")
            if dbg else None)

    SHSZ = DIM * TOK                       # one core's k shard, elems
    VROW = 16 * 65                         # v row: 16 heads x (64 chans + 1)
    SHSZV = TOK * VROW                     # one core's augmented v shard
    agk_in = nc.dram_tensor("agk_in", [SHSZ], BF)
    agk_out = nc.dram_tensor("agk_out", [4 * SHSZ], BF)
    agv_in = nc.dram_tensor("agv_in", [SHSZV], BF)
    agv_out = nc.dram_tensor("agv_out", [4 * SHSZV], BF)
    sescr = nc.dram_tensor("sescr", [1, 4 * TOK], BF)   # recip bcast scratch

    RG = [[0, 1, 2, 3], [4, 5, 6, 7]]

    with tile.TileContext(nc) as tc, ExitStack() as CTX:
        # ---------------- persistent SBUF ----------------
        pp = CTX.enter_context(tc.tile_pool(name="persist", bufs=1))
        qhat = [pp.tile([128, TOK], BF, tag=f"qhat{c}", name=f"qhat{c}")
                for c in range(8)]
        ktf = [pp.tile([128, 8, TOK], BF, tag=f"ktf{r}", name=f"ktf{r}")
               for r in range(4)]
        vaug = [[pp.tile([128, 4, VROW // 2], BF, tag=f"vaug{r}_{h}",
                         name=f"vaug{r}_{h}") for h in range(2)]
                for r in range(4)]
        attnT = [pp.tile([128, TOK], BF, tag=f"attnT{c}", name=f"attnT{c}")
                 for c in range(8)]

        bias_sb = pp.tile([1, 3 * DIM], BF, tag="bias_sb", name="bias_sb")
        wsum_sb = pp.tile([128, 8, 2], BF, tag="wsum_sb", name="wsum_sb")
        swp_sb = pp.tile([128, 128], BF, tag="swp_sb", name="swp_sb")
        ones_t = pp.tile([1, TOK], BF, tag="ones_t", name="ones_t")
        ones_c = pp.tile([1, 128], BF, tag="ones_c", name="ones_c")
        eps_sb = pp.tile([1, 1], F32, tag="eps_sb", name="eps_sb")
        tab_r = pp.tile([128, 4, TOK], BF, tag="tab_r", name="tab_r")
        tab_f = pp.tile([128, 4, TOK], BF, tag="tab_f", name="tab_f")

        nc.vector.memset(ones_t[:], 1.0)
        nc.vector.memset(ones_c[:], 1.0)
        nc.vector.memset(eps_sb[:], EPS)

        wo_res_pool = CTX.enter_context(tc.tile_pool(name="wores", bufs=1))

        # ---------------- phase 1: qkv + norm + rope + AG ----------------
        p1 = ExitStack()
        w_pool = p1.enter_context(tc.tile_pool(name="wls", bufs=2))
        sq_pool = p1.enter_context(tc.tile_pool(name="sq", bufs=4))
        qtsb_pool = p1.enter_context(tc.tile_pool(name="qtsb", bufs=12))
        scr1_pool = p1.enter_context(tc.tile_pool(name="scr1", bufs=4))
        rstd_pool = p1.enter_context(tc.tile_pool(name="rstd", bufs=4))
        # PSUM budget (8 banks): qkvp 3 + swpp 3 + ssqp 1 + bcp 1 = 8
        qkvp = p1.enter_context(tc.tile_pool(name="qkvp", bufs=2, space="PSUM"))
        vpsp = p1.enter_context(tc.tile_pool(name="vpsp", bufs=2, space="PSUM"))
        swpp = p1.enter_context(tc.tile_pool(name="swpp", bufs=2, space="PSUM"))
        ssqp = p1.enter_context(tc.tile_pool(name="ssqp", bufs=1, space="PSUM"))
        bcp = p1.enter_context(tc.tile_pool(name="bcp", bufs=1, space="PSUM"))

        misc1_pool = p1.enter_context(tc.tile_pool(name="misc1", bufs=1))
        xt_sb = misc1_pool.tile([128, 8, TOK], BF, tag="xt_sb", name="xt_sb")
        khat = [misc1_pool.tile([128, 4, TOK], BF, tag=f"khat{h}",
                                name=f"khat{h}") for h in range(2)]
        vloc = [misc1_pool.tile([128, 4, VROW // 2], BF, tag=f"vloc{h}",
                                name=f"vloc{h}") for h in range(2)]

        # prefetches: Wk's first column chunk, then x, then the rest of
        # Wk in growing pieces, so the first QKV matmul starts ~4.5us in
        wkt = w_pool.tile([128, 8, DIM], BF, tag="wt", name="wt1")
        for lo, hi in ((0, 128), (None, None), (128, 512), (512, 1024)):
            if lo is None:
                nc.sync.dma_start(
                    out=xt_sb[:],
                    in_=xT.ap().rearrange("(c p) t -> p c t", p=128))
                continue
            nc.sync.dma_start(
                out=wkt[:, :, lo:hi],
                in_=wqkv.ap()[:, DIM + lo:DIM + hi].rearrange(
                    "(c p) m -> p c m", p=128),
            )
        nc.scalar.dma_start(
            out=tab_r[:], in_=tabs.ap().rearrange("j p t -> p j t"))
        nc.scalar.dma_start(out=swp_sb[:], in_=swp.ap())
        nc.scalar.dma_start(
            out=wsum_sb[:], in_=wsum.ap().rearrange("(c p) w -> p c w", p=128))
        if not zero_bias:
            nc.scalar.dma_start(out=bias_sb[:], in_=bqkv.ap())

        def load_w(which):
            """which 0/1/2 -> q/k/v weight block as [128, 8, 1024]."""
            wt = w_pool.tile([128, 8, DIM], BF, tag="wt", name=f"wt{which}")
            nc.sync.dma_start(
                out=wt[:],
                in_=wqkv.ap()[:, which * DIM:(which + 1) * DIM].rearrange(
                    "(c p) m -> p c m", p=128),
            )
            return wt

        wk = wkt
        wq = load_w(0)

        HS = SHSZ // 2

        def store_k_half(h, eng):
            # AllGather-k input, split by head halves: the first half (k
            # chunks 0-3 = heads 0-7) flows store->copy->reload on the
            # gpsimd queue so the score stream starts while the rest moves.
            eng.dma_start(
                out=bass.AP(tensor=agk_in.ap().tensor, offset=h * HS,
                            ap=[[TOK, 128], [128 * TOK, 4], [1, TOK]]),
                in_=khat[h][:],
            )

        def qkv_chunk(wt, which, c, ssq):
            """One 128-channel chunk: psum matmuls, ACT evict, square+ssq."""
            ps = qkvp.tile([128, TOK], F32, tag="qkvps", name="qkvps")
            for ci in range(8):
                nc.tensor.matmul(
                    ps[:], wt[:, ci, c * 128:(c + 1) * 128], xt_sb[:, ci, :],
                    start=(ci == 0), stop=(zero_bias and ci == 7),
                )
            if not zero_bias:
                nc.tensor.matmul(
                    ps[:], bias_sb[:, which * DIM + c * 128:
                                   which * DIM + (c + 1) * 128],
                    ones_t[:], start=False, stop=True,
                )
            qt = qtsb_pool.tile([128, TOK], BF, tag="qt", name="qt")
            nc.scalar.copy(qt[:], ps[:])   # evict on ACT (idle in phase 1)
            sqt = sq_pool.tile([128, TOK], BF, tag="sqt", name="sqt")
            nc.vector.tensor_mul(sqt[:], qt[:], qt[:])
            return qt, sqt

        def rstd_tabs(which, ssq):
            # rstd = exp(-0.5 * ln(var + eps)); fold into the rope tables
            lnv = rstd_pool.tile([1, TOK], F32, tag="lnv", name="lnv")
            nc.scalar.activation(lnv[:], ssq[:], AF.Ln, bias=eps_sb[:])
            rstd = rstd_pool.tile([1, TOK], BF, tag="rstd", name="rstd")
            nc.scalar.activation(rstd[:], lnv[:], AF.Exp, scale=-0.5)
            bc = bcp.tile([128, TOK], F32, tag="bc", name="bc")
            nc.tensor.matmul(bc[:], ones_c[:], rstd[:], start=True, stop=True)
            for j in range(2):
                nc.vector.tensor_mul(
                    tab_f[:, 2 * which + j, :], tab_r[:, 2 * which + j, :],
                    bc[:])

        def rope_chunk(which, qt, dst):
            """dst = qt * C' + (P @ qt) * S'."""
            sw = swpp.tile([128, TOK], F32, tag="sw", name="sw")
            nc.tensor.matmul(sw[:], swp_sb[:], qt[:], start=True, stop=True)
            m1 = scr1_pool.tile([128, TOK], BF, tag="m1", name="m1")
            nc.vector.tensor_mul(m1[:], qt[:], tab_f[:, 2 * which, :])
            m2 = scr1_pool.tile([128, TOK], BF, tag="m2", name="m2")
            nc.vector.tensor_mul(m2[:], sw[:], tab_f[:, 2 * which + 1, :])
            nc.vector.tensor_add(dst, m1[:], m2[:])

        def emit_ssq(ssq, which, c, sqt):
            nc.tensor.matmul(
                ssq[:], wsum_sb[:, c, which:which + 1], sqt[:],
                start=(c == 0), stop=(c == 7),
            )

        # ---- k chunks (ssq matmuls lag 2 chunks so PE never waits the
        # evict->square chain)
        ssq_k = ssqp.tile([1, TOK], F32, tag="ssq", name="ssq_k")
        kt, ksq = [], []
        for c in range(8):
            a, b = qkv_chunk(wk, 1, c, ssq_k)
            kt.append(a)
            ksq.append(b)
            if c >= 2:
                emit_ssq(ssq_k, 1, c - 2, ksq[c - 2])
        emit_ssq(ssq_k, 1, 6, ksq[6])
        emit_ssq(ssq_k, 1, 7, ksq[7])
        rstd_tabs(1, ssq_k)
        # ---- q chunks with k-rope interleaved (PE never idles on the drain)
        wv = load_w(2)      # recycles a w buffer once the k-chunks finish
        ssq_q = ssqp.tile([1, TOK], F32, tag="ssq", name="ssq_q")
        qt, qsq = [], []
        for c in range(8):
            a, b = qkv_chunk(wq, 0, c, ssq_q)
            qt.append(a)
            qsq.append(b)
            if c >= 2:
                emit_ssq(ssq_q, 0, c - 2, qsq[c - 2])
            if c < 4:   # front-load the k-rope: 2 chunks per q-chunk
                rope_chunk(1, kt[2 * c], khat[c // 2][:, (2 * c) % 4, :])
                rope_chunk(1, kt[2 * c + 1], khat[c // 2][:, (2 * c + 1) % 4, :])
                if c == 1:
                    store_k_half(0, nc.gpsimd)
                if c == 3:
                    store_k_half(1, nc.sync)
        emit_ssq(ssq_q, 0, 6, qsq[6])
        emit_ssq(ssq_q, 0, 7, qsq[7])
        rstd_tabs(0, ssq_q)


        def agk_copy(r, h, eng):
            eng.dma_start(
                out=bass.AP(tensor=agk_out.ap().tensor,
                            offset=r * SHSZ + h * HS, ap=[[1, HS]]),
                in_=bass.AP(tensor=agk_in.ap().tensor, offset=h * HS,
                            ap=[[1, HS]]),
            )

        def ktf_reload(r, h, eng):
            srcap = bass.AP(
                tensor=agk_out.ap().tensor, offset=r * SHSZ + h * HS,
                ap=[[TOK, 128], [128 * TOK, 4], [1, TOK]],
            )
            eng.dma_start(out=ktf[r][:, 4 * h:4 * h + 4, :], in_=srcap)

        if single:
            for r in range(4):
                agk_copy(r, 0, nc.gpsimd)
                ktf_reload(r, 0, nc.gpsimd if r < 2 else nc.sync)
            for r in range(4):
                agk_copy(r, 1, nc.sync)
                ktf_reload(r, 1, nc.gpsimd if r % 2 == 0 else nc.sync)
        else:
            nc.gpsimd.collective_compute(
                "AllGather", mybir.AluOpType.bypass, replica_groups=RG,
                ins=[agk_in.ap().opt()], outs=[agk_out.ap().opt()],
            )
            for r in range(4):
                ktf_reload(r, 0, nc.gpsimd if r < 2 else nc.sync)
                ktf_reload(r, 1, nc.gpsimd if r % 2 == 0 else nc.sync)

        # ---- v chunks with q-rope interleaved, nh-outer so the heads-0-7
        # half of the augmented V completes (and gathers) first.  vloc
        # carries the (64 chans + ones) per-head layout so the gather
        # delivers AV-ready tiles with no strided reload.
        vlr = [vloc[h][:].rearrange("p f (g c) -> p f g c", c=65)
               for h in range(2)]
        for h in range(2):
            nc.vector.memset(vlr[h][:, :, :, 64:65], 1.0)

        def store_v_half(h, eng):
            eng.dma_start(
                out=bass.AP(tensor=agv_in.ap().tensor, offset=h * (VROW // 2),
                            ap=[[VROW, 128], [128 * VROW, 4], [1, VROW // 2]]),
                in_=vloc[h][:],
            )

        rope_chunk(0, qt[0], qhat[0][:])
        vi = 0
        for nh in range(2):
            for t4 in range(4):
                ps = vpsp.tile([128, TOK], F32, tag="vps", name="vps")
                for ci in range(8):
                    nc.tensor.matmul(
                        ps[:],
                        xt_sb[:, ci, t4 * 128:(t4 + 1) * 128],
                        wv[:, ci, nh * TOK:(nh + 1) * TOK],
                        start=(ci == 0), stop=(zero_bias and ci == 7),
                    )
                if not zero_bias:
                    nc.tensor.matmul(
                        ps[:], ones_c[:],
                        bias_sb[:, 2 * DIM + nh * TOK:2 * DIM + (nh + 1) * TOK],
                        start=False, stop=True,
                    )
                dst = vlr[nh][:, t4, :, 0:64]
                nc.vector.tensor_copy(dst, ps[:])
                if vi < 7:
                    rope_chunk(0, qt[vi + 1], qhat[vi + 1][:])
                vi += 1
            store_v_half(nh, nc.sync if nh == 0 else nc.sync)
        wo_res = wo_res_pool.tile([128, 8, DIN], BF, tag="wores", name="wores")
        nc.sync.dma_start(
            out=wo_res[:],
            in_=wout.ap().rearrange("(c p) m -> p c m", p=128),
        )

        def agv_copy(r, h, eng):
            eng.dma_start(
                out=bass.AP(tensor=agv_out.ap().tensor,
                            offset=r * SHSZV + h * (VROW // 2),
                            ap=[[VROW, TOK], [1, VROW // 2]]),
                in_=bass.AP(tensor=agv_in.ap().tensor, offset=h * (VROW // 2),
                            ap=[[VROW, TOK], [1, VROW // 2]]),
            )

        def vaug_reload(r, h, eng):
            srcap = bass.AP(
                tensor=agv_out.ap().tensor, offset=r * SHSZV + h * (VROW // 2),
                ap=[[VROW, 128], [128 * VROW, 4], [1, VROW // 2]],
            )
            eng.dma_start(out=vaug[r][h][:], in_=srcap)

        if single:
            for r in range(4):
                agv_copy(r, 0, nc.gpsimd)
                vaug_reload(r, 0, nc.gpsimd if r < 2 else nc.sync)
            for r in range(4):
                agv_copy(r, 1, nc.sync)
                vaug_reload(r, 1, nc.gpsimd if r % 2 == 0 else nc.sync)
        else:
            nc.gpsimd.collective_compute(
                "AllGather", mybir.AluOpType.bypass, replica_groups=RG,
                ins=[agv_in.ap().opt()], outs=[agv_out.ap().opt()],
            )
            for r in range(4):
                vaug_reload(r, 0, nc.gpsimd if r < 2 else nc.sync)
                vaug_reload(r, 1, nc.gpsimd if r % 2 == 0 else nc.sync)

        def dump8(tiles, cols=None):
            dmp = ExitStack()
            dp = dmp.enter_context(tc.tile_pool(name="dump", bufs=2))
            for c, t in enumerate(tiles):
                f = dp.tile([t.shape[0], TOK], F32, tag="dmp", name="dmp")
                srcap = t[:, cols] if cols is not None else t[:]
                nc.vector.tensor_copy(f[:], srcap)
                nc.gpsimd.dma_start(
                    out=dbgt.ap()[c * 128:c * 128 + t.shape[0], :], in_=f[:])
            dmp.close()

        if dbg == "qt":
            dump8(qt)
        if dbg == "qhat":
            dump8(qhat)
        if dbg == "khat":
            dump8([khat[c // 4][:, c % 4, :] for c in range(8)])
        if dbg == "ktf":
            dump8([ktf[0][:, c, 0:TOK] for c in range(8)])
        if dbg == "vaug":
            dump8([vaug[r][0][:, 0, 0:TOK] for r in range(4)] * 2)

        p1.close()

        # ---------------- phase 2: attention ----------------
        p2 = ExitStack()
        wo_pool = CTX.enter_context(tc.tile_pool(name="wo", bufs=3))
        # PSUM: scp 2x2 banks + avp 4x1 = 8
        scp = CTX.enter_context(tc.tile_pool(name="scp", bufs=2, space="PSUM"))
        avp = p2.enter_context(tc.tile_pool(name="avp", bufs=4, space="PSUM"))
        expt_pool = p2.enter_context(
            tc.tile_pool(name="expt", bufs=AVLAG + 2))
        nrm_pool = p2.enter_context(tc.tile_pool(name="nrm", bufs=2))
        ascr_pool = p2.enter_context(tc.tile_pool(name="ascr", bufs=4))

        av_tiles = {}
        expt = {}

        def emit_score_exp(hg, kc):
            sc = scp.tile([128, 2 * TOK], F32, tag="sc", name="sc")
            for hh in range(2):
                h = hg * 2 + hh
                nc.tensor.matmul(
                    sc[:, hh * TOK:(hh + 1) * TOK],
                    ktf[kc // 4][(h % 2) * 64:(h % 2) * 64 + 64, h // 2,
                                 (kc % 4) * 128:(kc % 4 + 1) * 128],
                    qhat[h // 2][(h % 2) * 64:(h % 2) * 64 + 64, :],
                    start=True, stop=True,
                )
            e = expt_pool.tile([128, 2 * TOK], BF, tag="expt", name="expt")
            nc.scalar.activation(e[:], sc[:], AF.Exp)
            expt[(hg, kc)] = e

        def emit_av(hg, j):
            e = expt.pop((hg, j))
            for hh in range(2):
                if j == 0:
                    av_tiles[(hg, hh)] = avp.tile(
                        [65, TOK], F32, tag="av", name="av")
                ha = hg * 2 + hh
                nc.tensor.matmul(
                    av_tiles[(hg, hh)][:],
                    vaug[j // 4][ha // 8][:, j % 4,
                                 (ha % 8) * 65:(ha % 8 + 1) * 65],
                    e[:, hh * TOK:(hh + 1) * TOK],
                    start=(j == 0), stop=(j == KC - 1),
                )

        def emit_normalize(hg):
            # sumexp rows live at PSUM partition 64; stage both heads' rows
            # into one SBUF row, reciprocal in place, round-trip through DRAM
            # once to broadcast across 64 partitions (stride-0 load), then
            # scale into the attnT slots (odd heads cross a partition offset,
            # which only a DMA can do).
            se = nrm_pool.tile([65, 2 * TOK], F32, tag="se", name="se")
            for hh in range(2):
                nc.vector.tensor_copy(
                    se[64:65, hh * TOK:(hh + 1) * TOK],
                    av_tiles[(hg, hh)][64:65, :],
                )
            nc.vector.reciprocal(out=se[64:65, :], in_=se[64:65, :])
            nc.gpsimd.dma_start(
                out=bass.AP(tensor=sescr.ap().tensor,
                            offset=(hg % 2) * 2 * TOK,
                            ap=[[1, 2 * TOK]]),
                in_=se[64:65, :])   # f32 -> bf16 casting DMA
            rbct = nrm_pool.tile([64, 2 * TOK], BF, tag="rbc", name="rbc")
            bcast_src = bass.AP(
                tensor=sescr.ap().tensor, offset=(hg % 2) * 2 * TOK,
                ap=[[0, 64], [1, 2 * TOK]],
            )
            nc.gpsimd.dma_start(out=rbct[:], in_=bcast_src)
            rbc = rbct[:]
            for hh in range(2):
                h = hg * 2 + hh
                if h % 2 == 0:
                    nc.vector.tensor_mul(
                        attnT[h // 2][0:64, :],
                        av_tiles[(hg, hh)][0:64, :],
                        rbc[0:64, hh * TOK:(hh + 1) * TOK],
                    )
                else:
                    a = ascr_pool.tile([64, TOK], BF, tag="ascr", name="ascr")
                    nc.vector.tensor_mul(
                        a[:], av_tiles[(hg, hh)][0:64, :],
                        rbc[0:64, hh * TOK:(hh + 1) * TOK],
                    )
                    nc.sync.dma_start(
                        out=attnT[h // 2][64:128, :], in_=a[:])

        # Score/exp stream runs ahead; the AV stream trails by AVLAG tiles
        # (covering the V-gather) and catches up after CATCHUP so the stream
        # drains straight into the output projection.
        NT = NHG2 * KC
        av_done = 0

        def av_target(t):
            base = max(0, t - AVLAG + 1)
            extra = max(0, (t - CATCHUP)) // 3
            if t >= NT - 1:
                return NT
            return min(NT, base + extra, t + 1)

        for t in range(NT):
            hg, j = t // KC, t % KC
            emit_score_exp(hg, j)
            while av_done < av_target(t):
                ahg, aj = av_done // KC, av_done % KC
                emit_av(ahg, aj)
                if aj == KC - 1:
                    emit_normalize(ahg)
                av_done += 1
        p2.close()

        # ---------------- phase 3: output projection ----------------
        # Two co-chunks share one PSUM tile; the head-pair whose attnT slot
        # lands last (ci=7, via hg7's normalize) is contracted last.
        CIORD = [0, 1, 2, 3, 4, 5, 6, 7]
        for cop in range(4):
            ps = scp.tile([128, 2 * TOK], F32, tag="sc", name="outps")
            for sub in range(2):
                co = cop * 2 + sub
                for i, ci in enumerate(CIORD):
                    nc.tensor.matmul(
                        ps[:, sub * TOK:(sub + 1) * TOK],
                        wo_res[:, ci, co * 128:(co + 1) * 128],
                        attnT[ci][:],
                        start=(i == 0), stop=(i == 7),
                    )
            for sub in range(2):
                co = cop * 2 + sub
                osb = wo_pool.tile([128, TOK], F32, tag="osb", name="osb")
                nc.vector.tensor_copy(osb[:], ps[:, sub * TOK:(sub + 1) * TOK])
                nc.sync.dma_start(out=out.ap()[co * 128:(co + 1) * 128, :],
                                  in_=osb[:])

    nc.compile()
    return nc


def _host_prep(inputs):
    import ml_dtypes

    bf16 = ml_dtypes.bfloat16
    x = np.asarray(inputs["x"], np.float32)
    Wqkv = np.asarray(inputs["Wqkv"], np.float32)
    bqkv = np.asarray(inputs["bqkv"], np.float32)
    qs = np.asarray(inputs["q_scale"], np.float32)
    ks = np.asarray(inputs["k_scale"], np.float32)
    Wout = np.asarray(inputs["Wout"], np.float32)

    p64 = np.concatenate([np.arange(0, 64, 2), np.arange(1, 64, 2)])
    perm = np.concatenate([64 * h + p64 for h in range(H)])

    qsp, ksp = qs[perm], ks[perm]
    Wq = Wqkv[:, :DIM][:, perm] * qsp[None, :]
    Wk = Wqkv[:, DIM:2 * DIM][:, perm] * ksp[None, :]
    Wv = Wqkv[:, 2 * DIM:]
    W = np.concatenate([Wq, Wk, Wv], 1).astype(bf16)
    bq = bqkv[:DIM][perm] * qsp
    bk = bqkv[DIM:2 * DIM][perm] * ksp
    bias = np.concatenate([bq, bk, bqkv[2 * DIM:]])[None, :].astype(bf16)
    wsum = np.stack(
        [1.0 / (DIM * qsp ** 2), 1.0 / (DIM * ksp ** 2)], 1
    ).astype(bf16)

    sw = np.arange(128)
    swap = np.where(sw % 64 < 32, sw + 32, sw - 32)
    P = np.zeros((128, 128), np.float32)
    P[swap, np.arange(128)] = 1.0  # (P.T @ x)[m] = x[swap[m]]
    P = P.astype(bf16)

    inv_freq = 1.0 / (BASE ** (np.arange(0, HD, 2).astype(np.float32) / HD))
    pos = np.maximum(np.arange(N) - 1, 0).astype(np.float32)
    ang = pos[:, None] * inv_freq[None, :]
    cosT, sinT = np.cos(ang).T, np.sin(ang).T           # (32, N)
    C128 = np.tile(cosT, (4, 1))                         # (128, N)
    S128 = np.concatenate([-sinT, sinT, -sinT, sinT], 0)

    in_maps = []
    for core in range(NCORE):
        b, sh = core // 4, core % 4
        t0 = sh * TOK
        xTs = np.ascontiguousarray(x[b, t0:t0 + TOK, :].T).astype(bf16)
        tabs = np.stack([
            C128[:, t0:t0 + TOK] * 0.125,
            S128[:, t0:t0 + TOK] * 0.125,
            C128[:, t0:t0 + TOK],
            S128[:, t0:t0 + TOK],
        ]).astype(bf16)
        in_maps.append({
            "xT": xTs,
            "wqkv": W,
            "bqkv": bias,
            "wsum": wsum,
            "swp": P,
            "tabs": np.ascontiguousarray(tabs),
            "wout": Wout.astype(bf16),
        })
    return in_maps


LAST_EXEC_NS = None


def kernel(**inputs):
    global LAST_EXEC_NS
    import os
    from concourse.bass_utils import run_bass_kernel_spmd

    dbg = os.environ.get("KERNEL_DBG") or None
    zb = bool(np.all(np.asarray(inputs["bqkv"]) == 0))
    key = f"nc{dbg}{zb}"
    if key not in _CACHE:
        _CACHE[key] = _build_nc(dbg, zero_bias=zb)
    nc = _CACHE[key]

    in_maps = _host_prep(inputs)
    trace = bool(int(os.environ.get("KERNEL_TRACE", "0")))
    tmpdir = None
    if trace:
        import tempfile
        import concourse.bass_utils as _bu
        _bu.upload_artifacts = lambda d: d  # keep artifacts local
        tmpdir = tempfile.mkdtemp(prefix="ktrace_")
        print("TRACE DIR:", tmpdir)
    res = run_bass_kernel_spmd(
        nc, in_maps, core_ids=list(range(NCORE)), trace=trace, tmpdir=tmpdir
    )
    LAST_EXEC_NS = res.exec_time_ns
    bout = np.asarray(inputs["bout"], np.float32)
    outf = np.empty((B, N, DIN), np.float32)
    for core in range(NCORE):
        b, sh = core // 4, core % 4
        t0 = sh * TOK
        outf[b, t0:t0 + TOK, :] = res.results[core]["out"].T
    outf += bout[None, None, :]
    return outf


def kernel_raw(inputs):
    """Debug helper: run and return the per-core raw [1024, 512] outputs."""
    global LAST_EXEC_NS
    import os
    from concourse.bass_utils import run_bass_kernel_spmd

    dbg = os.environ.get("KERNEL_DBG") or None
    zb = bool(np.all(np.asarray(inputs["bqkv"]) == 0))
    key = f"nc{dbg}{zb}"
    if key not in _CACHE:
        _CACHE[key] = _build_nc(dbg, zero_bias=zb)
    nc = _CACHE[key]
    in_maps = _host_prep(inputs)
    res = run_bass_kernel_spmd(nc, in_maps, core_ids=list(range(NCORE)))
    LAST_EXEC_NS = res.exec_time_ns
    key = "dbg" if dbg else "out"
    return [r[key] for r in res.results]


# revision 48
# speedup vs baseline: 1.2180x; 1.0286x over previous
"""Trainium2 8-core Bass kernel for nn_Attention_7112465842253.

Token-sharded attention: 512 tokens/core (cores 0-3 = batch 0, 4-7 = batch 1).
Per core, all-bf16 matmuls: QKV projection with q/k in transposed [chan, tok]
layout and v in [tok, chan]; RMSNorm via weighted-sumsq matmul with the Ln/Exp
rstd folded into the RoPE tables; RoPE as x*C + (P@x)*S with a PE
partition-swap matmul, the swaps interleaved into the next stream's QKV
chunks so the PE never stalls on the rope drain.  K is projected+roped first
and its AllGather launched early; Q next (it gates the score stream); V last
with a second AllGather that only gates AV.  Attention runs in scoresT layout
(k-tokens on partitions; softmax denominator from a ones column in V; Exp
fused into the PSUM->SBUF eviction on ScalarE).  The Exp stream on the scalar
engine is the critical path: the AV stream trails it by a lag that covers the
V-gather latency and catches up before the stream ends, so the kernel drains
straight into the output projection.  Host does layout prep and reassembly.
"""

import numpy as np

B, N, DIN, DIM, H, HD = 2, 2048, 1024, 1024, 16, 64
NCORE = 8
TOK = 512
EPS = 1e-6
BASE = 10000.0
KC = N // 128        # 16 k-token chunks
NHG2 = 8             # head-groups of 2
AVLAG = 20           # exp-tiles the AV stream initially trails by
CATCHUP = 60         # stream position where AV starts catching up

_CACHE = {}


def _build_nc(dbg=None, single=False, zero_bias=False):
    import concourse.bass as bass
    import concourse.tile as tile
    from concourse import bacc, mybir
    from contextlib import ExitStack

    BF = mybir.dt.bfloat16
    F32 = mybir.dt.float32
    AF = mybir.ActivationFunctionType

    nc = bacc.Bacc(
        "TRN2", target_bir_lowering=False, debug=False,
        num_devices=(1 if single else NCORE),
    )

    # ---------------- DRAM parameters ----------------
    xT = nc.dram_tensor("xT", [DIN, TOK], BF, kind="ExternalInput")
    wqkv = nc.dram_tensor("wqkv", [DIN, 3 * DIM], BF, kind="ExternalInput")
    bqkv = nc.dram_tensor("bqkv", [1, 3 * DIM], BF, kind="ExternalInput")
    wsum = nc.dram_tensor("wsum", [DIN, 2], BF, kind="ExternalInput")
    swp = nc.dram_tensor("swp", [128, 128], BF, kind="ExternalInput")
    tabs = nc.dram_tensor("tabs", [4, 128, TOK], BF, kind="ExternalInput")
    wout = nc.dram_tensor("wout", [DIM, DIN], BF, kind="ExternalInput")
    out = nc.dram_tensor("out", [DIN, TOK], F32, kind="ExternalOutput")
    dbgt = (nc.dram_tensor("dbg", [DIN, TOK], F32, kind="ExternalOutput")
            if dbg else None)

    SHSZ = DIM * TOK                       # one core's k shard, elems
    VROW = 16 * 65                         # v row: 16 heads x (64 chans + 1)
    SHSZV = TOK * VROW                     # one core's augmented v shard
    agk_in = nc.dram_tensor("agk_in", [SHSZ], BF)
    agk_out = nc.dram_tensor("agk_out", [4 * SHSZ], BF)
    agv_in = nc.dram_tensor("agv_in", [SHSZV], BF)
    agv_out = nc.dram_tensor("agv_out", [4 * SHSZV], BF)
    sescr = nc.dram_tensor("sescr", [1, 4 * TOK], BF)   # recip bcast scratch

    RG = [[0, 1, 2, 3], [4, 5, 6, 7]]

    with tile.TileContext(nc) as tc, ExitStack() as CTX:
        # ---------------- persistent SBUF ----------------
        pp = CTX.enter_context(tc.tile_pool(name="persist", bufs=1))
        qhat = [pp.tile([128, TOK], BF, tag=f"qhat{c}", name=f"qhat{c}")
                for c in range(8)]
        ktf = [pp.tile([128, 8, TOK], BF, tag=f"ktf{r}", name=f"ktf{r}")
               for r in range(4)]
        vaug = [[pp.tile([128, 4, VROW // 2], BF, tag=f"vaug{r}_{h}",
                         name=f"vaug{r}_{h}") for h in range(2)]
                for r in range(4)]
        attnT = [pp.tile([128, TOK], BF, tag=f"attnT{c}", name=f"attnT{c}")
                 for c in range(8)]

        bias_sb = pp.tile([1, 3 * DIM], BF, tag="bias_sb", name="bias_sb")
        wsum_sb = pp.tile([128, 8, 2], BF, tag="wsum_sb", name="wsum_sb")
        swp_sb = pp.tile([128, 128], BF, tag="swp_sb", name="swp_sb")
        ones_t = pp.tile([1, TOK], BF, tag="ones_t", name="ones_t")
        ones_c = pp.tile([1, 128], BF, tag="ones_c", name="ones_c")
        eps_sb = pp.tile([1, 1], F32, tag="eps_sb", name="eps_sb")
        tab_r = pp.tile([128, 4, TOK], BF, tag="tab_r", name="tab_r")
        tab_f = pp.tile([128, 4, TOK], BF, tag="tab_f", name="tab_f")

        nc.vector.memset(ones_t[:], 1.0)
        nc.vector.memset(ones_c[:], 1.0)
        nc.vector.memset(eps_sb[:], EPS)

        wo_res_pool = CTX.enter_context(tc.tile_pool(name="wores", bufs=1))

        # ---------------- phase 1: qkv + norm + rope + AG ----------------
        p1 = ExitStack()
        w_pool = p1.enter_context(tc.tile_pool(name="wls", bufs=2))
        sq_pool = p1.enter_context(tc.tile_pool(name="sq", bufs=4))
        qtsb_pool = p1.enter_context(tc.tile_pool(name="qtsb", bufs=12))
        scr1_pool = p1.enter_context(tc.tile_pool(name="scr1", bufs=4))
        rstd_pool = p1.enter_context(tc.tile_pool(name="rstd", bufs=4))
        # PSUM budget (8 banks): qkvp 3 + swpp 3 + ssqp 1 + bcp 1 = 8
        qkvp = p1.enter_context(tc.tile_pool(name="qkvp", bufs=2, space="PSUM"))
        vpsp = p1.enter_context(tc.tile_pool(name="vpsp", bufs=2, space="PSUM"))
        swpp = p1.enter_context(tc.tile_pool(name="swpp", bufs=2, space="PSUM"))
        ssqp = p1.enter_context(tc.tile_pool(name="ssqp", bufs=1, space="PSUM"))
        bcp = p1.enter_context(tc.tile_pool(name="bcp", bufs=1, space="PSUM"))

        misc1_pool = p1.enter_context(tc.tile_pool(name="misc1", bufs=1))
        xt_sb = misc1_pool.tile([128, 8, TOK], BF, tag="xt_sb", name="xt_sb")
        khat = [misc1_pool.tile([128, 4, TOK], BF, tag=f"khat{h}",
                                name=f"khat{h}") for h in range(2)]
        vloc = [misc1_pool.tile([128, 4, VROW // 2], BF, tag=f"vloc{h}",
                                name=f"vloc{h}") for h in range(2)]

        # prefetches: Wk's first column chunk, then x, then the rest of
        # Wk in growing pieces, so the first QKV matmul starts ~4.5us in
        wkt = w_pool.tile([128, 8, DIM], BF, tag="wt", name="wt1")
        for lo, hi in ((0, 128), (None, None), (128, 512), (512, 1024)):
            if lo is None:
                nc.sync.dma_start(
                    out=xt_sb[:],
                    in_=xT.ap().rearrange("(c p) t -> p c t", p=128))
                continue
            nc.sync.dma_start(
                out=wkt[:, :, lo:hi],
                in_=wqkv.ap()[:, DIM + lo:DIM + hi].rearrange(
                    "(c p) m -> p c m", p=128),
            )
        nc.scalar.dma_start(
            out=tab_r[:], in_=tabs.ap().rearrange("j p t -> p j t"))
        nc.scalar.dma_start(out=swp_sb[:], in_=swp.ap())
        nc.scalar.dma_start(
            out=wsum_sb[:], in_=wsum.ap().rearrange("(c p) w -> p c w", p=128))
        if not zero_bias:
            nc.scalar.dma_start(out=bias_sb[:], in_=bqkv.ap())

        def load_w(which):
            """which 0/1/2 -> q/k/v weight block as [128, 8, 1024]."""
            wt = w_pool.tile([128, 8, DIM], BF, tag="wt", name=f"wt{which}")
            nc.sync.dma_start(
                out=wt[:],
                in_=wqkv.ap()[:, which * DIM:(which + 1) * DIM].rearrange(
                    "(c p) m -> p c m", p=128),
            )
            return wt

        wk = wkt
        wq = load_w(0)

        HS = SHSZ // 2

        def store_k_half(h, eng):
            # AllGather-k input, split by head halves: the first half (k
            # chunks 0-3 = heads 0-7) flows store->copy->reload on the
            # gpsimd queue so the score stream starts while the rest moves.
            eng.dma_start(
                out=bass.AP(tensor=agk_in.ap().tensor, offset=h * HS,
                            ap=[[TOK, 128], [128 * TOK, 4], [1, TOK]]),
                in_=khat[h][:],
            )

        def qkv_chunk(wt, which, c, ssq):
            """One 128-channel chunk: psum matmuls, ACT evict, square+ssq."""
            ps = qkvp.tile([128, TOK], F32, tag="qkvps", name="qkvps")
            for ci in range(8):
                nc.tensor.matmul(
                    ps[:], wt[:, ci, c * 128:(c + 1) * 128], xt_sb[:, ci, :],
                    start=(ci == 0), stop=(zero_bias and ci == 7),
                )
            if not zero_bias:
                nc.tensor.matmul(
                    ps[:], bias_sb[:, which * DIM + c * 128:
                                   which * DIM + (c + 1) * 128],
                    ones_t[:], start=False, stop=True,
                )
            qt = qtsb_pool.tile([128, TOK], BF, tag="qt", name="qt")
            nc.scalar.copy(qt[:], ps[:])   # evict on ACT (idle in phase 1)
            sqt = sq_pool.tile([128, TOK], BF, tag="sqt", name="sqt")
            nc.vector.tensor_mul(sqt[:], qt[:], qt[:])
            return qt, sqt

        def rstd_tabs(which, ssq):
            # rstd = exp(-0.5 * ln(var + eps)); fold into the rope tables
            lnv = rstd_pool.tile([1, TOK], F32, tag="lnv", name="lnv")
            nc.scalar.activation(lnv[:], ssq[:], AF.Ln, bias=eps_sb[:])
            rstd = rstd_pool.tile([1, TOK], BF, tag="rstd", name="rstd")
            nc.scalar.activation(rstd[:], lnv[:], AF.Exp, scale=-0.5)
            bc = bcp.tile([128, TOK], F32, tag="bc", name="bc")
            nc.tensor.matmul(bc[:], ones_c[:], rstd[:], start=True, stop=True)
            for j in range(2):
                nc.vector.tensor_mul(
                    tab_f[:, 2 * which + j, :], tab_r[:, 2 * which + j, :],
                    bc[:])

        def rope_chunk(which, qt, dst):
            """dst = qt * C' + (P @ qt) * S'."""
            sw = swpp.tile([128, TOK], F32, tag="sw", name="sw")
            nc.tensor.matmul(sw[:], swp_sb[:], qt[:], start=True, stop=True)
            m1 = scr1_pool.tile([128, TOK], BF, tag="m1", name="m1")
            nc.vector.tensor_mul(m1[:], qt[:], tab_f[:, 2 * which, :])
            m2 = scr1_pool.tile([128, TOK], BF, tag="m2", name="m2")
            nc.vector.tensor_mul(m2[:], sw[:], tab_f[:, 2 * which + 1, :])
            nc.vector.tensor_add(dst, m1[:], m2[:])

        def emit_ssq(ssq, which, c, sqt):
            nc.tensor.matmul(
                ssq[:], wsum_sb[:, c, which:which + 1], sqt[:],
                start=(c == 0), stop=(c == 7),
            )

        # ---- k chunks (ssq matmuls lag 2 chunks so PE never waits the
        # evict->square chain)
        ssq_k = ssqp.tile([1, TOK], F32, tag="ssq", name="ssq_k")
        kt, ksq = [], []
        for c in range(8):
            a, b = qkv_chunk(wk, 1, c, ssq_k)
            kt.append(a)
            ksq.append(b)
            if c >= 2:
                emit_ssq(ssq_k, 1, c - 2, ksq[c - 2])
        emit_ssq(ssq_k, 1, 6, ksq[6])
        emit_ssq(ssq_k, 1, 7, ksq[7])
        rstd_tabs(1, ssq_k)
        # ---- q chunks with k-rope interleaved (PE never idles on the drain)
        wv = load_w(2)      # recycles a w buffer once the k-chunks finish
        ssq_q = ssqp.tile([1, TOK], F32, tag="ssq", name="ssq_q")
        qt, qsq = [], []
        for c in range(8):
            a, b = qkv_chunk(wq, 0, c, ssq_q)
            qt.append(a)
            qsq.append(b)
            if c >= 2:
                emit_ssq(ssq_q, 0, c - 2, qsq[c - 2])
            if c < 4:   # front-load the k-rope: 2 chunks per q-chunk
                rope_chunk(1, kt[2 * c], khat[c // 2][:, (2 * c) % 4, :])
                rope_chunk(1, kt[2 * c + 1], khat[c // 2][:, (2 * c + 1) % 4, :])
                if c == 1:
                    store_k_half(0, nc.gpsimd)
                if c == 3:
                    store_k_half(1, nc.sync)
        emit_ssq(ssq_q, 0, 6, qsq[6])
        emit_ssq(ssq_q, 0, 7, qsq[7])
        rstd_tabs(0, ssq_q)


        def agk_copy(r, h, eng):
            eng.dma_start(
                out=bass.AP(tensor=agk_out.ap().tensor,
                            offset=r * SHSZ + h * HS, ap=[[1, HS]]),
                in_=bass.AP(tensor=agk_in.ap().tensor, offset=h * HS,
                            ap=[[1, HS]]),
            )

        def ktf_reload(r, h, eng):
            srcap = bass.AP(
                tensor=agk_out.ap().tensor, offset=r * SHSZ + h * HS,
                ap=[[TOK, 128], [128 * TOK, 4], [1, TOK]],
            )
            eng.dma_start(out=ktf[r][:, 4 * h:4 * h + 4, :], in_=srcap)

        if single:
            for r in range(4):
                agk_copy(r, 0, nc.gpsimd)
                ktf_reload(r, 0, nc.gpsimd if r < 2 else nc.sync)
            for r in range(4):
                agk_copy(r, 1, nc.sync)
                ktf_reload(r, 1, nc.gpsimd if r % 2 == 0 else nc.sync)
        else:
            nc.gpsimd.collective_compute(
                "AllGather", mybir.AluOpType.bypass, replica_groups=RG,
                ins=[agk_in.ap().opt()], outs=[agk_out.ap().opt()],
            )
            for r in range(4):
                ktf_reload(r, 0, nc.gpsimd if r < 2 else nc.sync)
                ktf_reload(r, 1, nc.gpsimd if r % 2 == 0 else nc.sync)

        # ---- v chunks with q-rope interleaved, nh-outer so the heads-0-7
        # half of the augmented V completes (and gathers) first.  vloc
        # carries the (64 chans + ones) per-head layout so the gather
        # delivers AV-ready tiles with no strided reload.
        vlr = [vloc[h][:].rearrange("p f (g c) -> p f g c", c=65)
               for h in range(2)]
        for h in range(2):
            nc.vector.memset(vlr[h][:, :, :, 64:65], 1.0)

        def store_v_half(h, eng):
            eng.dma_start(
                out=bass.AP(tensor=agv_in.ap().tensor, offset=h * (VROW // 2),
                            ap=[[VROW, 128], [128 * VROW, 4], [1, VROW // 2]]),
                in_=vloc[h][:],
            )

        rope_chunk(0, qt[0], qhat[0][:])
        vi = 0
        for nh in range(2):
            for t4 in range(4):
                ps = vpsp.tile([128, TOK], F32, tag="vps", name="vps")
                for ci in range(8):
                    nc.tensor.matmul(
                        ps[:],
                        xt_sb[:, ci, t4 * 128:(t4 + 1) * 128],
                        wv[:, ci, nh * TOK:(nh + 1) * TOK],
                        start=(ci == 0), stop=(zero_bias and ci == 7),
                    )
                if not zero_bias:
                    nc.tensor.matmul(
                        ps[:], ones_c[:],
                        bias_sb[:, 2 * DIM + nh * TOK:2 * DIM + (nh + 1) * TOK],
                        start=False, stop=True,
                    )
                dst = vlr[nh][:, t4, :, 0:64]
                nc.vector.tensor_copy(dst, ps[:])
                if vi < 7:
                    rope_chunk(0, qt[vi + 1], qhat[vi + 1][:])
                vi += 1
            store_v_half(nh, nc.sync if nh == 0 else nc.sync)
        wo_res = wo_res_pool.tile([128, 8, DIN], BF, tag="wores", name="wores")
        nc.sync.dma_start(
            out=wo_res[:],
            in_=wout.ap().rearrange("(c p) m -> p c m", p=128),
        )

        def agv_copy(r, h, eng):
            eng.dma_start(
                out=bass.AP(tensor=agv_out.ap().tensor,
                            offset=r * SHSZV + h * (VROW // 2),
                            ap=[[VROW, TOK], [1, VROW // 2]]),
                in_=bass.AP(tensor=agv_in.ap().tensor, offset=h * (VROW // 2),
                            ap=[[VROW, TOK], [1, VROW // 2]]),
            )

        def vaug_reload(r, h, eng):
            srcap = bass.AP(
                tensor=agv_out.ap().tensor, offset=r * SHSZV + h * (VROW // 2),
                ap=[[VROW, 128], [128 * VROW, 4], [1, VROW // 2]],
            )
            eng.dma_start(out=vaug[r][h][:], in_=srcap)

        if single:
            for r in range(4):
                agv_copy(r, 0, nc.gpsimd)
                vaug_reload(r, 0, nc.gpsimd if r < 2 else nc.sync)
            for r in range(4):
                agv_copy(r, 1, nc.sync)
                vaug_reload(r, 1, nc.gpsimd if r % 2 == 0 else nc.sync)
        else:
            nc.gpsimd.collective_compute(
                "AllGather", mybir.AluOpType.bypass, replica_groups=RG,
                ins=[agv_in.ap().opt()], outs=[agv_out.ap().opt()],
            )
            for r in range(4):
                vaug_reload(r, 0, nc.gpsimd if r < 2 else nc.sync)
                vaug_reload(r, 1, nc.gpsimd if r % 2 == 0 else nc.sync)

        def dump8(tiles, cols=None):
            dmp = ExitStack()
            dp = dmp.enter_context(tc.tile_pool(name="dump", bufs=2))
            for c, t in enumerate(tiles):
                f = dp.tile([t.shape[0], TOK], F32, tag="dmp", name="dmp")
                srcap = t[:, cols] if cols is not None else t[:]
                nc.vector.tensor_copy(f[:], srcap)
                nc.gpsimd.dma_start(
                    out=dbgt.ap()[c * 128:c * 128 + t.shape[0], :], in_=f[:])
            dmp.close()

        if dbg == "qt":
            dump8(qt)
        if dbg == "qhat":
            dump8(qhat)
        if dbg == "khat":
            dump8([khat[c // 4][:, c % 4, :] for c in range(8)])
        if dbg == "ktf":
            dump8([ktf[0][:, c, 0:TOK] for c in range(8)])
        if dbg == "vaug":
            dump8([vaug[r][0][:, 0, 0:TOK] for r in range(4)] * 2)

        p1.close()

        # ---------------- phase 2: attention ----------------
        p2 = ExitStack()
        wo_pool = CTX.enter_context(tc.tile_pool(name="wo", bufs=3))
        # PSUM: scp 2x2 banks + avp 4x1 = 8
        scp = CTX.enter_context(tc.tile_pool(name="scp", bufs=2, space="PSUM"))
        avp = p2.enter_context(tc.tile_pool(name="avp", bufs=4, space="PSUM"))
        expt_pool = p2.enter_context(
            tc.tile_pool(name="expt", bufs=AVLAG + 2))
        nrm_pool = p2.enter_context(tc.tile_pool(name="nrm", bufs=2))
        ascr_pool = p2.enter_context(tc.tile_pool(name="ascr", bufs=4))

        av_tiles = {}
        expt = {}

        def emit_score_exp(hg, kc):
            sc = scp.tile([128, 2 * TOK], F32, tag="sc", name="sc")
            for hh in range(2):
                h = hg * 2 + hh
                nc.tensor.matmul(
                    sc[:, hh * TOK:(hh + 1) * TOK],
                    ktf[kc // 4][(h % 2) * 64:(h % 2) * 64 + 64, h // 2,
                                 (kc % 4) * 128:(kc % 4 + 1) * 128],
                    qhat[h // 2][(h % 2) * 64:(h % 2) * 64 + 64, :],
                    start=True, stop=True,
                )
            e = expt_pool.tile([128, 2 * TOK], BF, tag="expt", name="expt")
            nc.scalar.activation(e[:], sc[:], AF.Exp)
            expt[(hg, kc)] = e

        def emit_av(hg, j):
            e = expt.pop((hg, j))
            for hh in range(2):
                if j == 0:
                    av_tiles[(hg, hh)] = avp.tile(
                        [65, TOK], F32, tag="av", name="av")
                ha = hg * 2 + hh
                nc.tensor.matmul(
                    av_tiles[(hg, hh)][:],
                    vaug[j // 4][ha // 8][:, j % 4,
                                 (ha % 8) * 65:(ha % 8 + 1) * 65],
                    e[:, hh * TOK:(hh + 1) * TOK],
                    start=(j == 0), stop=(j == KC - 1),
                )

        def emit_normalize(hg):
            # sumexp rows live at PSUM partition 64; stage both heads' rows
            # into one SBUF row, reciprocal in place, round-trip through DRAM
            # once to broadcast across 64 partitions (stride-0 load), then
            # scale into the attnT slots (odd heads cross a partition offset,
            # which only a DMA can do).
            se = nrm_pool.tile([65, 2 * TOK], F32, tag="se", name="se")
            for hh in range(2):
                nc.vector.tensor_copy(
                    se[64:65, hh * TOK:(hh + 1) * TOK],
                    av_tiles[(hg, hh)][64:65, :],
                )
            nc.vector.reciprocal(out=se[64:65, :], in_=se[64:65, :])
            nc.gpsimd.dma_start(
                out=bass.AP(tensor=sescr.ap().tensor,
                            offset=(hg % 2) * 2 * TOK,
                            ap=[[1, 2 * TOK]]),
                in_=se[64:65, :])   # f32 -> bf16 casting DMA
            rbct = nrm_pool.tile([64, 2 * TOK], BF, tag="rbc", name="rbc")
            bcast_src = bass.AP(
                tensor=sescr.ap().tensor, offset=(hg % 2) * 2 * TOK,
                ap=[[0, 64], [1, 2 * TOK]],
            )
            nc.gpsimd.dma_start(out=rbct[:], in_=bcast_src)
            rbc = rbct[:]
            for hh in range(2):
                h = hg * 2 + hh
                if h % 2 == 0:
                    nc.vector.tensor_mul(
                        attnT[h // 2][0:64, :],
                        av_tiles[(hg, hh)][0:64, :],
                        rbc[0:64, hh * TOK:(hh + 1) * TOK],
                    )
                else:
                    a = ascr_pool.tile([64, TOK], BF, tag="ascr", name="ascr")
                    nc.vector.tensor_mul(
                        a[:], av_tiles[(hg, hh)][0:64, :],
                        rbc[0:64, hh * TOK:(hh + 1) * TOK],
                    )
                    nc.sync.dma_start(
                        out=attnT[h // 2][64:128, :], in_=a[:])

        # Score/exp stream runs ahead; the AV stream trails by AVLAG tiles
        # (covering the V-gather) and catches up after CATCHUP so the stream
        # drains straight into the output projection.
        NT = NHG2 * KC
        av_done = 0

        def av_target(t):
            base = max(0, t - AVLAG + 1)
            extra = max(0, (t - CATCHUP)) // 3
            if t >= NT - 1:
                return NT
            return min(NT, base + extra, t + 1)

        for t in range(NT):
            hg, j = t // KC, t % KC
            emit_score_exp(hg, j)
            while av_done < av_target(t):
                ahg, aj = av_done // KC, av_done % KC
                emit_av(ahg, aj)
                if aj == KC - 1:
                    emit_normalize(ahg)
                av_done += 1
        # ---------------- phase 3: output projection ----------------
        # Two co-chunks share one PSUM tile.  attnT[7] lands last (hg7's
        # normalize), so the first two co-pairs accumulate ci 0-6 during that
        # window and only the ci=7 contraction waits for it.
        def op_partial(ps, sub, co, cis, first, last):
            for i, ci in enumerate(cis):
                nc.tensor.matmul(
                    ps[:, sub * TOK:(sub + 1) * TOK],
                    wo_res[:, ci, co * 128:(co + 1) * 128],
                    attnT[ci][:],
                    start=(first and i == 0), stop=(last and i == len(cis) - 1),
                )

        def op_finish(ps, cop):
            for sub in range(2):
                co = cop * 2 + sub
                osb = wo_pool.tile([128, TOK], F32, tag="osb", name="osb")
                nc.vector.tensor_copy(osb[:], ps[:, sub * TOK:(sub + 1) * TOK])
                nc.sync.dma_start(out=out.ap()[co * 128:(co + 1) * 128, :],
                                  in_=osb[:])

        early = []
        for cop in range(2):
            ps = scp.tile([128, 2 * TOK], F32, tag="sc", name="outps")
            for sub in range(2):
                op_partial(ps, sub, cop * 2 + sub, list(range(7)), True, False)
            early.append(ps)
        for cop in range(2):
            ps = early[cop]
            for sub in range(2):
                op_partial(ps, sub, cop * 2 + sub, [7], False, True)
            op_finish(ps, cop)
        for cop in range(2, 4):
            ps = scp.tile([128, 2 * TOK], F32, tag="sc", name="outps")
            for sub in range(2):
                op_partial(ps, sub, cop * 2 + sub, list(range(8)), True, True)
            op_finish(ps, cop)
        p2.close()

    nc.compile()
    return nc


def _host_prep(inputs):
    import ml_dtypes

    bf16 = ml_dtypes.bfloat16
    x = np.asarray(inputs["x"], np.float32)
    Wqkv = np.asarray(inputs["Wqkv"], np.float32)
    bqkv = np.asarray(inputs["bqkv"], np.float32)
    qs = np.asarray(inputs["q_scale"], np.float32)
    ks = np.asarray(inputs["k_scale"], np.float32)
    Wout = np.asarray(inputs["Wout"], np.float32)

    p64 = np.concatenate([np.arange(0, 64, 2), np.arange(1, 64, 2)])
    perm = np.concatenate([64 * h + p64 for h in range(H)])

    qsp, ksp = qs[perm], ks[perm]
    Wq = Wqkv[:, :DIM][:, perm] * qsp[None, :]
    Wk = Wqkv[:, DIM:2 * DIM][:, perm] * ksp[None, :]
    Wv = Wqkv[:, 2 * DIM:]
    W = np.concatenate([Wq, Wk, Wv], 1).astype(bf16)
    bq = bqkv[:DIM][perm] * qsp
    bk = bqkv[DIM:2 * DIM][perm] * ksp
    bias = np.concatenate([bq, bk, bqkv[2 * DIM:]])[None, :].astype(bf16)
    wsum = np.stack(
        [1.0 / (DIM * qsp ** 2), 1.0 / (DIM * ksp ** 2)], 1
    ).astype(bf16)

    sw = np.arange(128)
    swap = np.where(sw % 64 < 32, sw + 32, sw - 32)
    P = np.zeros((128, 128), np.float32)
    P[swap, np.arange(128)] = 1.0  # (P.T @ x)[m] = x[swap[m]]
    P = P.astype(bf16)

    inv_freq = 1.0 / (BASE ** (np.arange(0, HD, 2).astype(np.float32) / HD))
    pos = np.maximum(np.arange(N) - 1, 0).astype(np.float32)
    ang = pos[:, None] * inv_freq[None, :]
    cosT, sinT = np.cos(ang).T, np.sin(ang).T           # (32, N)
    C128 = np.tile(cosT, (4, 1))                         # (128, N)
    S128 = np.concatenate([-sinT, sinT, -sinT, sinT], 0)

    in_maps = []
    for core in range(NCORE):
        b, sh = core // 4, core % 4
        t0 = sh * TOK
        xTs = np.ascontiguousarray(x[b, t0:t0 + TOK, :].T).astype(bf16)
        tabs = np.stack([
            C128[:, t0:t0 + TOK] * 0.125,
            S128[:, t0:t0 + TOK] * 0.125,
            C128[:, t0:t0 + TOK],
            S128[:, t0:t0 + TOK],
        ]).astype(bf16)
        in_maps.append({
            "xT": xTs,
            "wqkv": W,
            "bqkv": bias,
            "wsum": wsum,
            "swp": P,
            "tabs": np.ascontiguousarray(tabs),
            "wout": Wout.astype(bf16),
        })
    return in_maps


LAST_EXEC_NS = None


def kernel(**inputs):
    global LAST_EXEC_NS
    import os
    from concourse.bass_utils import run_bass_kernel_spmd

    dbg = os.environ.get("KERNEL_DBG") or None
    zb = bool(np.all(np.asarray(inputs["bqkv"]) == 0))
    key = f"nc{dbg}{zb}"
    if key not in _CACHE:
        _CACHE[key] = _build_nc(dbg, zero_bias=zb)
    nc = _CACHE[key]

    in_maps = _host_prep(inputs)
    trace = bool(int(os.environ.get("KERNEL_TRACE", "0")))
    tmpdir = None
    if trace:
        import tempfile
        import concourse.bass_utils as _bu
        _bu.upload_artifacts = lambda d: d  # keep artifacts local
        tmpdir = tempfile.mkdtemp(prefix="ktrace_")
        print("TRACE DIR:", tmpdir)
    res = run_bass_kernel_spmd(
        nc, in_maps, core_ids=list(range(NCORE)), trace=trace, tmpdir=tmpdir
    )
    LAST_EXEC_NS = res.exec_time_ns
    bout = np.asarray(inputs["bout"], np.float32)
    outf = np.empty((B, N, DIN), np.float32)
    for core in range(NCORE):
        b, sh = core // 4, core % 4
        t0 = sh * TOK
        outf[b, t0:t0 + TOK, :] = res.results[core]["out"].T
    outf += bout[None, None, :]
    return outf


def kernel_raw(inputs):
    """Debug helper: run and return the per-core raw [1024, 512] outputs."""
    global LAST_EXEC_NS
    import os
    from concourse.bass_utils import run_bass_kernel_spmd

    dbg = os.environ.get("KERNEL_DBG") or None
    zb = bool(np.all(np.asarray(inputs["bqkv"]) == 0))
    key = f"nc{dbg}{zb}"
    if key not in _CACHE:
        _CACHE[key] = _build_nc(dbg, zero_bias=zb)
    nc = _CACHE[key]
    in_maps = _host_prep(inputs)
    res = run_bass_kernel_spmd(nc, in_maps, core_ids=list(range(NCORE)))
    LAST_EXEC_NS = res.exec_time_ns
    key = "dbg" if dbg else "out"
    return [r[key] for r in res.results]


# revision 52
# speedup vs baseline: 1.2188x; 1.0007x over previous
"""Trainium2 8-core Bass kernel for nn_Attention_7112465842253.

Token-sharded attention: 512 tokens/core (cores 0-3 = batch 0, 4-7 = batch 1).
Per core, all-bf16 matmuls: QKV projection with q/k in transposed [chan, tok]
layout and v in [tok, chan]; RMSNorm via weighted-sumsq matmul with the Ln/Exp
rstd folded into the RoPE tables; RoPE as x*C + (P@x)*S with a PE
partition-swap matmul, the swaps interleaved into the next stream's QKV
chunks so the PE never stalls on the rope drain.  K is projected+roped first
and its AllGather launched early; Q next (it gates the score stream); V last
with a second AllGather that only gates AV.  Attention runs in scoresT layout
(k-tokens on partitions; softmax denominator from a ones column in V; Exp
fused into the PSUM->SBUF eviction on ScalarE).  The Exp stream on the scalar
engine is the critical path: the AV stream trails it by a lag that covers the
V-gather latency and catches up before the stream ends, so the kernel drains
straight into the output projection.  Host does layout prep and reassembly.
"""

import numpy as np

B, N, DIN, DIM, H, HD = 2, 2048, 1024, 1024, 16, 64
NCORE = 8
TOK = 512
EPS = 1e-6
BASE = 10000.0
KC = N // 128        # 16 k-token chunks
NHG2 = 8             # head-groups of 2
AVLAG = 20           # exp-tiles the AV stream initially trails by
CATCHUP = 60         # stream position where AV starts catching up

_CACHE = {}


def _build_nc(dbg=None, single=False, zero_bias=False):
    import concourse.bass as bass
    import concourse.tile as tile
    from concourse import bacc, mybir
    from contextlib import ExitStack

    BF = mybir.dt.bfloat16
    F32 = mybir.dt.float32
    AF = mybir.ActivationFunctionType

    nc = bacc.Bacc(
        "TRN2", target_bir_lowering=False, debug=False,
        num_devices=(1 if single else NCORE),
    )

    # ---------------- DRAM parameters ----------------
    xT = nc.dram_tensor("xT", [DIN, TOK], BF, kind="ExternalInput")
    wqkv = nc.dram_tensor("wqkv", [DIN, 3 * DIM], BF, kind="ExternalInput")
    bqkv = nc.dram_tensor("bqkv", [1, 3 * DIM], BF, kind="ExternalInput")
    wsum = nc.dram_tensor("wsum", [DIN, 2], BF, kind="ExternalInput")
    swp = nc.dram_tensor("swp", [128, 128], BF, kind="ExternalInput")
    tabs = nc.dram_tensor("tabs", [4, 128, TOK], BF, kind="ExternalInput")
    wout = nc.dram_tensor("wout", [DIM, DIN], BF, kind="ExternalInput")
    out = nc.dram_tensor("out", [DIN, TOK], F32, kind="ExternalOutput")
    dbgt = (nc.dram_tensor("dbg", [DIN, TOK], F32, kind="ExternalOutput")
            if dbg else None)

    SHSZ = DIM * TOK                       # one core's k shard, elems
    VROW = 16 * 65                         # v row: 16 heads x (64 chans + 1)
    SHSZV = TOK * VROW                     # one core's augmented v shard
    agk_in = nc.dram_tensor("agk_in", [SHSZ], BF)
    agk_out = nc.dram_tensor("agk_out", [4 * SHSZ], BF)
    agv_in = nc.dram_tensor("agv_in", [SHSZV], BF)
    agv_out = nc.dram_tensor("agv_out", [4 * SHSZV], BF)
    sescr = nc.dram_tensor("sescr", [1, 4 * TOK], BF)   # recip bcast scratch

    RG = [[0, 1, 2, 3], [4, 5, 6, 7]]

    with tile.TileContext(nc) as tc, ExitStack() as CTX:
        # ---------------- persistent SBUF ----------------
        pp = CTX.enter_context(tc.tile_pool(name="persist", bufs=1))
        qhat = [pp.tile([128, TOK], BF, tag=f"qhat{c}", name=f"qhat{c}")
                for c in range(8)]
        ktf = [pp.tile([128, 8, TOK], BF, tag=f"ktf{r}", name=f"ktf{r}")
               for r in range(4)]
        vaug = [[pp.tile([128, 4, VROW // 2], BF, tag=f"vaug{r}_{h}",
                         name=f"vaug{r}_{h}") for h in range(2)]
                for r in range(4)]
        attnT = [pp.tile([128, TOK], BF, tag=f"attnT{c}", name=f"attnT{c}")
                 for c in range(8)]

        bias_sb = pp.tile([1, 3 * DIM], BF, tag="bias_sb", name="bias_sb")
        wsum_sb = pp.tile([128, 8, 2], BF, tag="wsum_sb", name="wsum_sb")
        swp_sb = pp.tile([128, 128], BF, tag="swp_sb", name="swp_sb")
        ones_t = pp.tile([1, TOK], BF, tag="ones_t", name="ones_t")
        ones_c = pp.tile([1, 128], BF, tag="ones_c", name="ones_c")
        eps_sb = pp.tile([1, 1], F32, tag="eps_sb", name="eps_sb")
        tab_r = pp.tile([128, 4, TOK], BF, tag="tab_r", name="tab_r")
        tab_f = pp.tile([128, 4, TOK], BF, tag="tab_f", name="tab_f")

        nc.vector.memset(ones_t[:], 1.0)
        nc.vector.memset(ones_c[:], 1.0)
        nc.vector.memset(eps_sb[:], EPS)

        wo_res_pool = CTX.enter_context(tc.tile_pool(name="wores", bufs=1))

        # ---------------- phase 1: qkv + norm + rope + AG ----------------
        p1 = ExitStack()
        w_pool = p1.enter_context(tc.tile_pool(name="wls", bufs=2))
        sq_pool = p1.enter_context(tc.tile_pool(name="sq", bufs=4))
        qtsb_pool = p1.enter_context(tc.tile_pool(name="qtsb", bufs=12))
        scr1_pool = p1.enter_context(tc.tile_pool(name="scr1", bufs=4))
        rstd_pool = p1.enter_context(tc.tile_pool(name="rstd", bufs=4))
        # PSUM budget (8 banks): qkvp 3 + swpp 3 + ssqp 1 + bcp 1 = 8
        qkvp = p1.enter_context(tc.tile_pool(name="qkvp", bufs=2, space="PSUM"))
        vpsp = p1.enter_context(tc.tile_pool(name="vpsp", bufs=2, space="PSUM"))
        swpp = p1.enter_context(tc.tile_pool(name="swpp", bufs=2, space="PSUM"))
        ssqp = p1.enter_context(tc.tile_pool(name="ssqp", bufs=1, space="PSUM"))
        bcp = p1.enter_context(tc.tile_pool(name="bcp", bufs=1, space="PSUM"))

        misc1_pool = p1.enter_context(tc.tile_pool(name="misc1", bufs=1))
        xt_sb = misc1_pool.tile([128, 8, TOK], BF, tag="xt_sb", name="xt_sb")
        khat = [misc1_pool.tile([128, 4, TOK], BF, tag=f"khat{h}",
                                name=f"khat{h}") for h in range(2)]
        vloc = [misc1_pool.tile([128, 4, VROW // 2], BF, tag=f"vloc{h}",
                                name=f"vloc{h}") for h in range(2)]

        # prefetches: Wk's first column chunk, then x, then the rest of
        # Wk in growing pieces, so the first QKV matmul starts ~4.5us in
        wkt = w_pool.tile([128, 8, DIM], BF, tag="wt", name="wt1")
        for lo, hi in ((0, 128), (None, None), (128, 512), (512, 1024)):
            if lo is None:
                nc.sync.dma_start(
                    out=xt_sb[:],
                    in_=xT.ap().rearrange("(c p) t -> p c t", p=128))
                continue
            nc.sync.dma_start(
                out=wkt[:, :, lo:hi],
                in_=wqkv.ap()[:, DIM + lo:DIM + hi].rearrange(
                    "(c p) m -> p c m", p=128),
            )
        nc.scalar.dma_start(
            out=tab_r[:], in_=tabs.ap().rearrange("j p t -> p j t"))
        nc.scalar.dma_start(out=swp_sb[:], in_=swp.ap())
        nc.scalar.dma_start(
            out=wsum_sb[:], in_=wsum.ap().rearrange("(c p) w -> p c w", p=128))
        if not zero_bias:
            nc.scalar.dma_start(out=bias_sb[:], in_=bqkv.ap())

        def load_w(which):
            """which 0/1/2 -> q/k/v weight block as [128, 8, 1024]."""
            wt = w_pool.tile([128, 8, DIM], BF, tag="wt", name=f"wt{which}")
            nc.sync.dma_start(
                out=wt[:],
                in_=wqkv.ap()[:, which * DIM:(which + 1) * DIM].rearrange(
                    "(c p) m -> p c m", p=128),
            )
            return wt

        wk = wkt
        wq = load_w(0)

        HS = SHSZ // 2

        def store_k_half(h, eng):
            # AllGather-k input, split by head halves: the first half (k
            # chunks 0-3 = heads 0-7) flows store->copy->reload on the
            # gpsimd queue so the score stream starts while the rest moves.
            eng.dma_start(
                out=bass.AP(tensor=agk_in.ap().tensor, offset=h * HS,
                            ap=[[TOK, 128], [128 * TOK, 4], [1, TOK]]),
                in_=khat[h][:],
            )

        def qkv_chunk(wt, which, c, ssq):
            """One 128-channel chunk: psum matmuls, ACT evict, square+ssq."""
            ps = qkvp.tile([128, TOK], F32, tag="qkvps", name="qkvps")
            for ci in range(8):
                nc.tensor.matmul(
                    ps[:], wt[:, ci, c * 128:(c + 1) * 128], xt_sb[:, ci, :],
                    start=(ci == 0), stop=(zero_bias and ci == 7),
                )
            if not zero_bias:
                nc.tensor.matmul(
                    ps[:], bias_sb[:, which * DIM + c * 128:
                                   which * DIM + (c + 1) * 128],
                    ones_t[:], start=False, stop=True,
                )
            qt = qtsb_pool.tile([128, TOK], BF, tag="qt", name="qt")
            nc.scalar.copy(qt[:], ps[:])   # evict on ACT (idle in phase 1)
            sqt = sq_pool.tile([128, TOK], BF, tag="sqt", name="sqt")
            nc.vector.tensor_mul(sqt[:], qt[:], qt[:])
            return qt, sqt

        def rstd_tabs(which, ssq):
            # rstd = exp(-0.5 * ln(var + eps)); fold into the rope tables
            lnv = rstd_pool.tile([1, TOK], F32, tag="lnv", name="lnv")
            nc.scalar.activation(lnv[:], ssq[:], AF.Ln, bias=eps_sb[:])
            rstd = rstd_pool.tile([1, TOK], BF, tag="rstd", name="rstd")
            nc.scalar.activation(rstd[:], lnv[:], AF.Exp, scale=-0.5)
            bc = bcp.tile([128, TOK], F32, tag="bc", name="bc")
            nc.tensor.matmul(bc[:], ones_c[:], rstd[:], start=True, stop=True)
            for j in range(2):
                nc.vector.tensor_mul(
                    tab_f[:, 2 * which + j, :], tab_r[:, 2 * which + j, :],
                    bc[:])

        def rope_chunk(which, qt, dst):
            """dst = qt * C' + (P @ qt) * S'."""
            sw = swpp.tile([128, TOK], F32, tag="sw", name="sw")
            nc.tensor.matmul(sw[:], swp_sb[:], qt[:], start=True, stop=True)
            m1 = scr1_pool.tile([128, TOK], BF, tag="m1", name="m1")
            nc.vector.tensor_mul(m1[:], qt[:], tab_f[:, 2 * which, :])
            m2 = scr1_pool.tile([128, TOK], BF, tag="m2", name="m2")
            nc.vector.tensor_mul(m2[:], sw[:], tab_f[:, 2 * which + 1, :])
            nc.vector.tensor_add(dst, m1[:], m2[:])

        def emit_ssq(ssq, which, c, sqt):
            nc.tensor.matmul(
                ssq[:], wsum_sb[:, c, which:which + 1], sqt[:],
                start=(c == 0), stop=(c == 7),
            )

        # ---- k chunks (ssq matmuls lag 2 chunks so PE never waits the
        # evict->square chain)
        ssq_k = ssqp.tile([1, TOK], F32, tag="ssq", name="ssq_k")
        kt, ksq = [], []
        for c in range(8):
            a, b = qkv_chunk(wk, 1, c, ssq_k)
            kt.append(a)
            ksq.append(b)
            if c >= 2:
                emit_ssq(ssq_k, 1, c - 2, ksq[c - 2])
        emit_ssq(ssq_k, 1, 6, ksq[6])
        emit_ssq(ssq_k, 1, 7, ksq[7])
        rstd_tabs(1, ssq_k)
        # ---- q chunks with k-rope interleaved (PE never idles on the drain)
        wv = load_w(2)      # recycles a w buffer once the k-chunks finish
        ssq_q = ssqp.tile([1, TOK], F32, tag="ssq", name="ssq_q")
        qt, qsq = [], []
        for c in range(8):
            a, b = qkv_chunk(wq, 0, c, ssq_q)
            qt.append(a)
            qsq.append(b)
            if c >= 2:
                emit_ssq(ssq_q, 0, c - 2, qsq[c - 2])
            if c < 4:   # front-load the k-rope: 2 chunks per q-chunk
                rope_chunk(1, kt[2 * c], khat[c // 2][:, (2 * c) % 4, :])
                rope_chunk(1, kt[2 * c + 1], khat[c // 2][:, (2 * c + 1) % 4, :])
                if c == 1:
                    store_k_half(0, nc.gpsimd)
                if c == 3:
                    store_k_half(1, nc.sync)
        emit_ssq(ssq_q, 0, 6, qsq[6])
        emit_ssq(ssq_q, 0, 7, qsq[7])
        rstd_tabs(0, ssq_q)


        def agk_copy(r, h, eng):
            eng.dma_start(
                out=bass.AP(tensor=agk_out.ap().tensor,
                            offset=r * SHSZ + h * HS, ap=[[1, HS]]),
                in_=bass.AP(tensor=agk_in.ap().tensor, offset=h * HS,
                            ap=[[1, HS]]),
            )

        def ktf_reload(r, h, eng):
            srcap = bass.AP(
                tensor=agk_out.ap().tensor, offset=r * SHSZ + h * HS,
                ap=[[TOK, 128], [128 * TOK, 4], [1, TOK]],
            )
            eng.dma_start(out=ktf[r][:, 4 * h:4 * h + 4, :], in_=srcap)

        if single:
            for r in range(4):
                agk_copy(r, 0, nc.gpsimd)
                ktf_reload(r, 0, nc.gpsimd if r < 2 else nc.sync)
            for r in range(4):
                agk_copy(r, 1, nc.sync)
                ktf_reload(r, 1, nc.gpsimd if r % 2 == 0 else nc.sync)
        else:
            nc.gpsimd.collective_compute(
                "AllGather", mybir.AluOpType.bypass, replica_groups=RG,
                ins=[agk_in.ap().opt()], outs=[agk_out.ap().opt()],
            )
            for r in range(4):
                ktf_reload(r, 0, nc.gpsimd if r < 2 else nc.sync)
                ktf_reload(r, 1, nc.gpsimd if r % 2 == 0 else nc.sync)

        # ---- v chunks with q-rope interleaved, nh-outer so the heads-0-7
        # half of the augmented V completes (and gathers) first.  vloc
        # carries the (64 chans + ones) per-head layout so the gather
        # delivers AV-ready tiles with no strided reload.
        vlr = [vloc[h][:].rearrange("p f (g c) -> p f g c", c=65)
               for h in range(2)]
        for h in range(2):
            nc.vector.memset(vlr[h][:, :, :, 64:65], 1.0)

        def store_v_half(h, eng):
            eng.dma_start(
                out=bass.AP(tensor=agv_in.ap().tensor, offset=h * (VROW // 2),
                            ap=[[VROW, 128], [128 * VROW, 4], [1, VROW // 2]]),
                in_=vloc[h][:],
            )

        rope_chunk(0, qt[0], qhat[0][:])
        vi = 0
        for nh in range(2):
            for t4 in range(4):
                ps = vpsp.tile([128, TOK], F32, tag="vps", name="vps")
                for ci in range(8):
                    nc.tensor.matmul(
                        ps[:],
                        xt_sb[:, ci, t4 * 128:(t4 + 1) * 128],
                        wv[:, ci, nh * TOK:(nh + 1) * TOK],
                        start=(ci == 0), stop=(zero_bias and ci == 7),
                    )
                if not zero_bias:
                    nc.tensor.matmul(
                        ps[:], ones_c[:],
                        bias_sb[:, 2 * DIM + nh * TOK:2 * DIM + (nh + 1) * TOK],
                        start=False, stop=True,
                    )
                dst = vlr[nh][:, t4, :, 0:64]
                nc.vector.tensor_copy(dst, ps[:])
                if vi < 7:
                    rope_chunk(0, qt[vi + 1], qhat[vi + 1][:])
                vi += 1
            store_v_half(nh, nc.sync if nh == 0 else nc.sync)
        wo_res = wo_res_pool.tile([128, 8, DIN], BF, tag="wores", name="wores")
        nc.sync.dma_start(
            out=wo_res[:],
            in_=wout.ap().rearrange("(c p) m -> p c m", p=128),
        )

        def agv_copy(r, h, eng):
            eng.dma_start(
                out=bass.AP(tensor=agv_out.ap().tensor,
                            offset=r * SHSZV + h * (VROW // 2),
                            ap=[[VROW, TOK], [1, VROW // 2]]),
                in_=bass.AP(tensor=agv_in.ap().tensor, offset=h * (VROW // 2),
                            ap=[[VROW, TOK], [1, VROW // 2]]),
            )

        def vaug_reload(r, h, eng):
            srcap = bass.AP(
                tensor=agv_out.ap().tensor, offset=r * SHSZV + h * (VROW // 2),
                ap=[[VROW, 128], [128 * VROW, 4], [1, VROW // 2]],
            )
            eng.dma_start(out=vaug[r][h][:], in_=srcap)

        if single:
            for r in range(4):
                agv_copy(r, 0, nc.gpsimd)
                vaug_reload(r, 0, nc.gpsimd if r < 2 else nc.sync)
            for r in range(4):
                agv_copy(r, 1, nc.sync)
                vaug_reload(r, 1, nc.gpsimd if r % 2 == 0 else nc.sync)
        else:
            nc.gpsimd.collective_compute(
                "AllGather", mybir.AluOpType.bypass, replica_groups=RG,
                ins=[agv_in.ap().opt()], outs=[agv_out.ap().opt()],
            )
            for r in range(4):
                vaug_reload(r, 0, nc.gpsimd if r < 2 else nc.sync)
                vaug_reload(r, 1, nc.gpsimd if r % 2 == 0 else nc.sync)

        def dump8(tiles, cols=None):
            dmp = ExitStack()
            dp = dmp.enter_context(tc.tile_pool(name="dump", bufs=2))
            for c, t in enumerate(tiles):
                f = dp.tile([t.shape[0], TOK], F32, tag="dmp", name="dmp")
                srcap = t[:, cols] if cols is not None else t[:]
                nc.vector.tensor_copy(f[:], srcap)
                nc.gpsimd.dma_start(
                    out=dbgt.ap()[c * 128:c * 128 + t.shape[0], :], in_=f[:])
            dmp.close()

        if dbg == "qt":
            dump8(qt)
        if dbg == "qhat":
            dump8(qhat)
        if dbg == "khat":
            dump8([khat[c // 4][:, c % 4, :] for c in range(8)])
        if dbg == "ktf":
            dump8([ktf[0][:, c, 0:TOK] for c in range(8)])
        if dbg == "vaug":
            dump8([vaug[r][0][:, 0, 0:TOK] for r in range(4)] * 2)

        p1.close()

        # ---------------- phase 2: attention ----------------
        p2 = ExitStack()
        wo_pool = CTX.enter_context(tc.tile_pool(name="wo", bufs=3))
        # PSUM: avp 4x1 + scp 2x2 banks = 8.  avp is created first so it
        # recycles the banks whose phase-1 readers finish last (v psums),
        # while scp lands on early-freed banks and the score stream can
        # start before the v chunks drain.
        avp = CTX.enter_context(tc.tile_pool(name="avp", bufs=4, space="PSUM"))
        scp = CTX.enter_context(tc.tile_pool(name="scp", bufs=2, space="PSUM"))
        expt_pool = p2.enter_context(
            tc.tile_pool(name="expt", bufs=AVLAG + 2))
        nrm_pool = p2.enter_context(tc.tile_pool(name="nrm", bufs=2))
        ascr_pool = p2.enter_context(tc.tile_pool(name="ascr", bufs=4))

        av_tiles = {}
        expt = {}

        def emit_score_exp(hg, kc):
            sc = scp.tile([128, 2 * TOK], F32, tag="sc", name="sc")
            for hh in range(2):
                h = hg * 2 + hh
                nc.tensor.matmul(
                    sc[:, hh * TOK:(hh + 1) * TOK],
                    ktf[kc // 4][(h % 2) * 64:(h % 2) * 64 + 64, h // 2,
                                 (kc % 4) * 128:(kc % 4 + 1) * 128],
                    qhat[h // 2][(h % 2) * 64:(h % 2) * 64 + 64, :],
                    start=True, stop=True,
                )
            e = expt_pool.tile([128, 2 * TOK], BF, tag="expt", name="expt")
            nc.scalar.activation(e[:], sc[:], AF.Exp)
            expt[(hg, kc)] = e

        def emit_av(hg, j):
            e = expt.pop((hg, j))
            for hh in range(2):
                if j == 0:
                    av_tiles[(hg, hh)] = avp.tile(
                        [65, TOK], F32, tag="av", name="av")
                ha = hg * 2 + hh
                nc.tensor.matmul(
                    av_tiles[(hg, hh)][:],
                    vaug[j // 4][ha // 8][:, j % 4,
                                 (ha % 8) * 65:(ha % 8 + 1) * 65],
                    e[:, hh * TOK:(hh + 1) * TOK],
                    start=(j == 0), stop=(j == KC - 1),
                )

        def emit_normalize(hg):
            # sumexp rows live at PSUM partition 64; stage both heads' rows
            # into one SBUF row, reciprocal in place, round-trip through DRAM
            # once to broadcast across 64 partitions (stride-0 load), then
            # scale into the attnT slots (odd heads cross a partition offset,
            # which only a DMA can do).
            se = nrm_pool.tile([65, 2 * TOK], F32, tag="se", name="se")
            for hh in range(2):
                nc.vector.tensor_copy(
                    se[64:65, hh * TOK:(hh + 1) * TOK],
                    av_tiles[(hg, hh)][64:65, :],
                )
            nc.vector.reciprocal(out=se[64:65, :], in_=se[64:65, :])
            nc.gpsimd.dma_start(
                out=bass.AP(tensor=sescr.ap().tensor,
                            offset=(hg % 2) * 2 * TOK,
                            ap=[[1, 2 * TOK]]),
                in_=se[64:65, :])   # f32 -> bf16 casting DMA
            rbct = nrm_pool.tile([64, 2 * TOK], BF, tag="rbc", name="rbc")
            bcast_src = bass.AP(
                tensor=sescr.ap().tensor, offset=(hg % 2) * 2 * TOK,
                ap=[[0, 64], [1, 2 * TOK]],
            )
            nc.gpsimd.dma_start(out=rbct[:], in_=bcast_src)
            rbc = rbct[:]
            for hh in range(2):
                h = hg * 2 + hh
                if h % 2 == 0:
                    nc.vector.tensor_mul(
                        attnT[h // 2][0:64, :],
                        av_tiles[(hg, hh)][0:64, :],
                        rbc[0:64, hh * TOK:(hh + 1) * TOK],
                    )
                else:
                    a = ascr_pool.tile([64, TOK], BF, tag="ascr", name="ascr")
                    nc.vector.tensor_mul(
                        a[:], av_tiles[(hg, hh)][0:64, :],
                        rbc[0:64, hh * TOK:(hh + 1) * TOK],
                    )
                    nc.sync.dma_start(
                        out=attnT[h // 2][64:128, :], in_=a[:])

        # Score/exp stream runs ahead; the AV stream trails by AVLAG tiles
        # (covering the V-gather) and catches up after CATCHUP so the stream
        # drains straight into the output projection.
        NT = NHG2 * KC
        av_done = 0

        def av_target(t):
            base = max(0, t - AVLAG + 1)
            extra = max(0, (t - CATCHUP)) // 3
            if t >= NT - 1:
                return NT
            return min(NT, base + extra, t + 1)

        for t in range(NT):
            hg, j = t // KC, t % KC
            emit_score_exp(hg, j)
            while av_done < av_target(t):
                ahg, aj = av_done // KC, av_done % KC
                emit_av(ahg, aj)
                if aj == KC - 1:
                    emit_normalize(ahg)
                av_done += 1
        # ---------------- phase 3: output projection ----------------
        # Two co-chunks share one PSUM tile.  attnT[7] lands last (hg7's
        # normalize), so the first two co-pairs accumulate ci 0-6 during that
        # window and only the ci=7 contraction waits for it.
        def op_partial(ps, sub, co, cis, first, last):
            for i, ci in enumerate(cis):
                nc.tensor.matmul(
                    ps[:, sub * TOK:(sub + 1) * TOK],
                    wo_res[:, ci, co * 128:(co + 1) * 128],
                    attnT[ci][:],
                    start=(first and i == 0), stop=(last and i == len(cis) - 1),
                )

        def op_finish(ps, cop):
            for sub in range(2):
                co = cop * 2 + sub
                osb = wo_pool.tile([128, TOK], F32, tag="osb", name="osb")
                nc.vector.tensor_copy(osb[:], ps[:, sub * TOK:(sub + 1) * TOK])
                nc.sync.dma_start(out=out.ap()[co * 128:(co + 1) * 128, :],
                                  in_=osb[:])

        early = []
        for cop in range(2):
            ps = scp.tile([128, 2 * TOK], F32, tag="sc", name="outps")
            for sub in range(2):
                op_partial(ps, sub, cop * 2 + sub, list(range(7)), True, False)
            early.append(ps)
        for cop in range(2):
            ps = early[cop]
            for sub in range(2):
                op_partial(ps, sub, cop * 2 + sub, [7], False, True)
            op_finish(ps, cop)
        for cop in range(2, 4):
            ps = scp.tile([128, 2 * TOK], F32, tag="sc", name="outps")
            for sub in range(2):
                op_partial(ps, sub, cop * 2 + sub, list(range(8)), True, True)
            op_finish(ps, cop)
        p2.close()

    nc.compile()
    return nc


def _host_prep(inputs):
    import ml_dtypes

    bf16 = ml_dtypes.bfloat16
    x = np.asarray(inputs["x"], np.float32)
    Wqkv = np.asarray(inputs["Wqkv"], np.float32)
    bqkv = np.asarray(inputs["bqkv"], np.float32)
    qs = np.asarray(inputs["q_scale"], np.float32)
    ks = np.asarray(inputs["k_scale"], np.float32)
    Wout = np.asarray(inputs["Wout"], np.float32)

    p64 = np.concatenate([np.arange(0, 64, 2), np.arange(1, 64, 2)])
    perm = np.concatenate([64 * h + p64 for h in range(H)])

    qsp, ksp = qs[perm], ks[perm]
    Wq = Wqkv[:, :DIM][:, perm] * qsp[None, :]
    Wk = Wqkv[:, DIM:2 * DIM][:, perm] * ksp[None, :]
    Wv = Wqkv[:, 2 * DIM:]
    W = np.concatenate([Wq, Wk, Wv], 1).astype(bf16)
    bq = bqkv[:DIM][perm] * qsp
    bk = bqkv[DIM:2 * DIM][perm] * ksp
    bias = np.concatenate([bq, bk, bqkv[2 * DIM:]])[None, :].astype(bf16)
    wsum = np.stack(
        [1.0 / (DIM * qsp ** 2), 1.0 / (DIM * ksp ** 2)], 1
    ).astype(bf16)

    sw = np.arange(128)
    swap = np.where(sw % 64 < 32, sw + 32, sw - 32)
    P = np.zeros((128, 128), np.float32)
    P[swap, np.arange(128)] = 1.0  # (P.T @ x)[m] = x[swap[m]]
    P = P.astype(bf16)

    inv_freq = 1.0 / (BASE ** (np.arange(0, HD, 2).astype(np.float32) / HD))
    pos = np.maximum(np.arange(N) - 1, 0).astype(np.float32)
    ang = pos[:, None] * inv_freq[None, :]
    cosT, sinT = np.cos(ang).T, np.sin(ang).T           # (32, N)
    C128 = np.tile(cosT, (4, 1))                         # (128, N)
    S128 = np.concatenate([-sinT, sinT, -sinT, sinT], 0)

    in_maps = []
    for core in range(NCORE):
        b, sh = core // 4, core % 4
        t0 = sh * TOK
        xTs = np.ascontiguousarray(x[b, t0:t0 + TOK, :].T).astype(bf16)
        tabs = np.stack([
            C128[:, t0:t0 + TOK] * 0.125,
            S128[:, t0:t0 + TOK] * 0.125,
            C128[:, t0:t0 + TOK],
            S128[:, t0:t0 + TOK],
        ]).astype(bf16)
        in_maps.append({
            "xT": xTs,
            "wqkv": W,
            "bqkv": bias,
            "wsum": wsum,
            "swp": P,
            "tabs": np.ascontiguousarray(tabs),
            "wout": Wout.astype(bf16),
        })
    return in_maps


LAST_EXEC_NS = None


def kernel(**inputs):
    global LAST_EXEC_NS
    import os
    from concourse.bass_utils import run_bass_kernel_spmd

    dbg = os.environ.get("KERNEL_DBG") or None
    zb = bool(np.all(np.asarray(inputs["bqkv"]) == 0))
    key = f"nc{dbg}{zb}"
    if key not in _CACHE:
        _CACHE[key] = _build_nc(dbg, zero_bias=zb)
    nc = _CACHE[key]

    in_maps = _host_prep(inputs)
    trace = bool(int(os.environ.get("KERNEL_TRACE", "0")))
    tmpdir = None
    if trace:
        import tempfile
        import concourse.bass_utils as _bu
        _bu.upload_artifacts = lambda d: d  # keep artifacts local
        tmpdir = tempfile.mkdtemp(prefix="ktrace_")
        print("TRACE DIR:", tmpdir)
    res = run_bass_kernel_spmd(
        nc, in_maps, core_ids=list(range(NCORE)), trace=trace, tmpdir=tmpdir
    )
    LAST_EXEC_NS = res.exec_time_ns
    bout = np.asarray(inputs["bout"], np.float32)
    outf = np.empty((B, N, DIN), np.float32)
    for core in range(NCORE):
        b, sh = core // 4, core % 4
        t0 = sh * TOK
        outf[b, t0:t0 + TOK, :] = res.results[core]["out"].T
    outf += bout[None, None, :]
    return outf


def kernel_raw(inputs):
    """Debug helper: run and return the per-core raw [1024, 512] outputs."""
    global LAST_EXEC_NS
    import os
    from concourse.bass_utils import run_bass_kernel_spmd

    dbg = os.environ.get("KERNEL_DBG") or None
    zb = bool(np.all(np.asarray(inputs["bqkv"]) == 0))
    key = f"nc{dbg}{zb}"
    if key not in _CACHE:
        _CACHE[key] = _build_nc(dbg, zero_bias=zb)
    nc = _CACHE[key]
    in_maps = _host_prep(inputs)
    res = run_bass_kernel_spmd(nc, in_maps, core_ids=list(range(NCORE)))
    LAST_EXEC_NS = res.exec_time_ns
    key = "dbg" if dbg else "out"
    return [r[key] for r in res.results]
